# revision 1
# baseline (speedup 1.0000x reference)
"""Trainium2 Bass kernel for nn_Network_67388036874689.

Data-parallel over batch: B=256 sharded as 32 samples on each of 8 cores;
all parameters replicated.

Structure exploited (validated against the reference on host):
  - fog_of_war's greedy scan returns arange(B) -> the permutation is identity.
  - conv2d(3x3, pad=1) on [C, H, 1] spatial input only sees kernel column 1
    -> 1D conv over H with 3 taps.
  - Embedding lookup (V=14) followed by pair-maxpool = lookup into a 196-entry
    pairwise-max table, implemented as one-hot matmuls on the PE.
  - The manipulator conv input is constant over H -> collapses to 3 matmuls
    (interior / h=0 / h=127 tap-sum variants).

Precision: critical path to the token discretization (enemy branch + manip)
in fp32 / float32r; post-token friend branch in bf16.
"""

import numpy as np
import ml_dtypes
from contextlib import ExitStack

import concourse.bass as bass
import concourse.bacc as bacc
import concourse.mybir as mybir
import concourse.tile as tile
from concourse.masks import make_identity
from concourse.bass_utils import run_bass_kernel_spmd

F32 = mybir.dt.float32
F32R = mybir.dt.float32r
BF16 = mybir.dt.bfloat16
I32 = mybir.dt.int32
AF = mybir.ActivationFunctionType
ALU = mybir.AluOpType
AX = mybir.AxisListType

NCORES = 8
B = 256
BC = B // NCORES        # 32 samples per core
L = 256                 # sequence length
V = 14                  # vocab
EMB = 512               # embedding dim
H = L // 2              # 128 pooled positions
NPAIR = V * V           # 196
PAIR0 = 112             # pair-table chunk split: 112 (t0 0..7) + 84 (t0 8..13)
PAIR1 = NPAIR - PAIR0   # 84
DEBUG_TAPS = False      # add intermediate DRAM outputs for debugging
SLAB = 8                # samples per embed/pool slab group
NGRP = BC // SLAB       # 4 groups
SLABW = SLAB * (H + 1) + 1   # padded slab width (stride 129 per sample)


def _dram_inputs(nc):
    t = {}

    def inp(name, shape, dt):
        t[name] = nc.dram_tensor(name, list(shape), dt, kind="ExternalInput").ap()

    inp("x", (BC, L), I32)
    inp("eemb", (V, EMB), F32R)
    inp("ecw", (256, 512 * 3), F32)       # enemy conv center col, [o, i*3+dh]
    inp("ecb", (256,), F32)
    inp("elw", (32768, 128), F32)
    inp("elb", (128,), F32)
    inp("mcw", (64, 128 * 3), F32)        # manip conv center col
    inp("mcb", (64,), F32)
    inp("mlw", (8192, 256), F32R)
    inp("mlb", (256,), F32)
    inp("femb", (V, EMB), BF16)
    inp("fcw", (256, 512 * 3), BF16)
    inp("fcb", (256,), F32)
    inp("flw", (32768, 128), BF16)
    inp("flb", (128,), F32)
    inp("f2w", (128, 14), F32)
    inp("f2b", (14,), F32)
    t["out"] = nc.dram_tensor("out", [BC, 14], F32, kind="ExternalOutput").ap()
    return t


def _tap(nc, io, name, ap):
    if not DEBUG_TAPS:
        return
    shape = list(ap.shape)
    t = nc.dram_tensor("tap_" + name, shape, ap.dtype, kind="ExternalOutput").ap()
    io["tap_" + name] = t
    nc.gpsimd.dma_start(t, ap)


def build_kernel(nc, tc, ctx):
    io = _dram_inputs(nc)
    consts = ctx.enter_context(tc.tile_pool(name="consts", bufs=1))
    work = ctx.enter_context(tc.tile_pool(name="work", bufs=1))
    wpool = ctx.enter_context(tc.tile_pool(name="wstream", bufs=8))
    prep = ctx.enter_context(tc.tile_pool(name="prep", bufs=2))
    psum_emb = ctx.enter_context(tc.tile_pool(name="psum_emb", bufs=4, space="PSUM"))
    psum_conv = ctx.enter_context(tc.tile_pool(name="psum_conv", bufs=2, space="PSUM"))
    psum_lin = ctx.enter_context(tc.tile_pool(name="psum_lin", bufs=1, space="PSUM"))
    psum_sm = ctx.enter_context(tc.tile_pool(name="psum_sm", bufs=1, space="PSUM"))

    def ctile(shape, dt, tag):
        return consts.tile(shape, dt, tag=tag, name=tag)

    def wtile(shape, dt, tag):
        return work.tile(shape, dt, tag=tag, name=tag)

    # ---------------- constants ----------------
    identF = ctile([128, 128], F32, "identF")
    make_identity(nc, identF)
    identB = ctile([128, 128], BF16, "identB")
    make_identity(nc, identB)
    iota_i = ctile([128, 1], I32, "iota_i")
    nc.gpsimd.iota(iota_i[:, :], pattern=[[0, 1]], base=0, channel_multiplier=1)
    iota_col = ctile([128, 1], F32, "iota_col")
    nc.vector.tensor_copy(iota_col[:, :], iota_i[:, :])
    ones_col = ctile([128, 1], F32, "ones_col")
    nc.vector.memset(ones_col[:, :], 1.0)
    ones_row = ctile([1, 128], F32, "ones_row")
    nc.vector.memset(ones_row[:, :], 1.0)
    iota_row = ctile([1, 128], F32, "iota_row")
    nc.gpsimd.dma_start(iota_row[:, :], iota_col[:, :])
    e0_row = ctile([1, 128], F32, "e0_row")
    nc.vector.tensor_scalar(e0_row[:, :], iota_row[:, :], 0.0, None, ALU.is_equal)
    eL_row = ctile([1, 128], F32, "eL_row")
    nc.vector.tensor_scalar(eL_row[:, :], iota_row[:, :], 127.0, None, ALU.is_equal)
    ei_row = ctile([1, 128], F32, "ei_row")
    nc.vector.scalar_tensor_tensor(ei_row[:, :], e0_row[:, :], -1.0, eL_row[:, :],
                                   ALU.mult, ALU.subtract)
    nc.vector.tensor_scalar(ei_row[:, :], ei_row[:, :], 1.0, None, ALU.add)
    zpad = ctile([128, 32], F32, "zpad")
    nc.vector.memset(zpad[:, :], 0.0)
    zpadr = ctile([128, 32], F32R, "zpadr")
    nc.vector.tensor_copy(zpadr[:, :], zpad[:, :])
    zpadb = ctile([128, 32], BF16, "zpadb")
    nc.vector.tensor_copy(zpadb[:, :], zpad[:, :])

    def bias_col(dram_vec, n, tag):
        col = ctile([n, 1], F32, tag)
        nc.gpsimd.dma_start(col[:, :], dram_vec)
        return col

    def bias_bcast(dram_vec, rows, width, tag):
        out = ctile([rows, width], F32, tag)
        nc.gpsimd.dma_start(out[:, :], dram_vec[None, :].partition_broadcast(rows))
        return out

    EBc = bias_bcast(io["ecb"], 128, 256, "EB")
    FBc = bias_bcast(io["fcb"], 128, 256, "FB")
    MBc = bias_bcast(io["mlb"], BC, 256, "MB")
    F2Bc = bias_bcast(io["f2b"], BC, 14, "F2B")
    elb_col = bias_col(io["elb"], 128, "elb")
    flb_col = bias_col(io["flb"], 128, "flb")
    mcb_col = bias_col(io["mcb"], 64, "mcb")

    # pair-max tables: pm[t0, t1*512+ch] = max(emb[t0,ch], emb[t1,ch]).
    # Built as two partition-base-0 pieces (t0 0..7 / 8..13), then reshaped
    # to [pair, ch] partition chunks by SBUF->SBUF DMA (all on-chip).
    def pair_table(emb_dram, dt, tag):
        embA = ctile([8, EMB], dt, tag + "_embA")
        nc.gpsimd.dma_start(embA[:, :], emb_dram[0:8, :])
        embB = ctile([6, EMB], dt, tag + "_embB")
        nc.gpsimd.dma_start(embB[:, :], emb_dram[8:V, :])
        tps = []
        for half, esb, nt0 in (("0", embA, 8), ("1", embB, 6)):
            pm = work.tile([nt0, V * EMB], dt, tag="pm", name="pm" + half)
            for t1 in range(V):
                embt1 = prep.tile([V, EMB], dt, tag="embt1", name="embt1")
                nc.gpsimd.dma_start(embt1[:, :],
                                  emb_dram[t1, :][None, :].partition_broadcast(V))
                nc.vector.tensor_tensor(pm[:, t1 * EMB:(t1 + 1) * EMB],
                                        esb[:, :], embt1[0:nt0, :], ALU.max)
            tp = ctile([nt0 * V, EMB], dt, tag + half)
            nc.gpsimd.dma_start(tp[:, :], pm[:, :])
            tps.append(tp)
        return tps[0], tps[1]

    tpE0, tpE1 = pair_table(io["eemb"], F32R, "tpE")
    _tap(nc, io, "tpE0", tpE0[:, :])
    _tap(nc, io, "tpE1", tpE1[:, :])
    tpF0, tpF1 = pair_table(io["femb"], BF16, "tpF")

    # conv weights -> 4 tiles [128 i, dh*256 + o] per branch
    def conv_wt(cw_dram, load_dt, wt_dt, ident, tag):
        wts = [ctile([128, 3 * 256], wt_dt, f"{tag}{kc}") for kc in range(4)]
        for oc in range(2):
            wsb = work.tile([128, 1536], load_dt, tag="pm", name="wsb")
            nc.gpsimd.dma_start(wsb[:, :], cw_dram[oc * 128:(oc + 1) * 128, :])
            for kc in range(4):
                for dh in range(3):
                    tp = psum_sm.tile([128, 128], load_dt, tag="sm", name="tpsum")
                    src = wsb[:, (kc * 128 * 3 + dh):((kc + 1) * 128 * 3):3]
                    nc.tensor.transpose(tp[:, :], src, ident)
                    nc.vector.tensor_copy(
                        wts[kc][:, dh * 256 + oc * 128: dh * 256 + (oc + 1) * 128],
                        tp[:, :])
        return wts

    wtE = conv_wt(io["ecw"], F32, F32R, identF, "wtE")
    wtF = conv_wt(io["fcw"], BF16, BF16, identB, "wtF")
    for kc in range(4):
        _tap(nc, io, f"wtE{kc}", wtE[kc][:, :])
        _tap(nc, io, f"wtF{kc}", wtF[kc][:, :])

    # manip tap-sum weights, transposed to [128 i, 64 o]
    wMsb = wtile([64, 384], F32, "wMsb")
    nc.gpsimd.dma_start(wMsb[:, :], io["mcw"])
    s01 = wtile([64, 128], F32, "s01")
    nc.vector.tensor_tensor(s01[:, :], wMsb[:, 0:384:3], wMsb[:, 1:384:3], ALU.add)
    s12 = wtile([64, 128], F32, "s12")
    nc.vector.tensor_tensor(s12[:, :], wMsb[:, 1:384:3], wMsb[:, 2:384:3], ALU.add)
    sint = wtile([64, 128], F32, "sint")
    nc.vector.tensor_tensor(sint[:, :], s01[:, :], wMsb[:, 2:384:3], ALU.add)
    wsumT = {}
    for name, src in (("int", sint), ("h0", s12), ("hL", s01)):
        tp = psum_sm.tile([128, 64], F32, tag="sm", name="tpsum")
        nc.tensor.transpose(tp[:, :], src[:, :], identF[0:64, 0:64])
        wsumT[name] = ctile([128, 64], F32R, f"wsumT_{name}")
        nc.vector.tensor_copy(wsumT[name][:, :], tp[:, :])

    # ---------------- shared stage helpers ----------------
    def embed_pool_grp(idx_row, g, tp0, tp1, slab_dt, tag):
        """Group g (8 samples): one-hot embed + pair-max -> 4 padded slabs."""
        slabs = [work.tile([128, SLABW], slab_dt, tag=f"slab{kc}",
                           name=f"slab{kc}") for kc in range(4)]
        zsrc = zpadb if slab_dt == BF16 else zpadr
        npad = SLAB + 1
        for kc in range(4):
            nc.vector.tensor_copy(slabs[kc][:, 0:SLABW:H + 1], zsrc[:, 0:npad])
        npos = SLAB * H  # 1024
        oh0 = work.tile([PAIR0, npos], slab_dt, tag="oh0", name="oh0")
        oh1 = work.tile([PAIR1, npos], slab_dt, tag="oh1", name="oh1")
        for nt in range(npos // 512):
            idxpp = psum_emb.tile([PAIR0, 512], F32, tag="pp", name="idxpp")
            nc.tensor.matmul(idxpp[:, :], ones_row[:, 0:PAIR0],
                             idx_row[:, g * npos + nt * 512:
                                     g * npos + (nt + 1) * 512],
                             start=True, stop=True)
            nc.vector.tensor_scalar(oh0[:, nt * 512:(nt + 1) * 512],
                                    idxpp[:, :], iota_col[0:PAIR0, :],
                                    None, ALU.is_equal)
            nc.vector.tensor_scalar(oh1[:, nt * 512:(nt + 1) * 512],
                                    idxpp[0:PAIR1, :], float(PAIR0),
                                    iota_col[0:PAIR1, :], ALU.subtract,
                                    ALU.is_equal)
        mm0, mm1, mo0, mo1 = tp0, tp1, oh0, oh1
        _tap(nc, io, f"{tag}_g{g}oh0", oh0[:, :])
        for kc in range(4):
            for nt in range(npos // 512):
                pp = psum_emb.tile([128, 512], F32, tag="pp", name="pp")
                nc.tensor.matmul(pp[:, :], mm0[:, kc * 128:(kc + 1) * 128],
                                 mo0[:, nt * 512:(nt + 1) * 512],
                                 start=True, stop=False)
                nc.tensor.matmul(pp[:, :], mm1[:, kc * 128:(kc + 1) * 128],
                                 mo1[:, nt * 512:(nt + 1) * 512],
                                 start=False, stop=True)
                # scatter 4 samples x 128 positions into the padded slab
                s0 = nt * 4
                dst = slabs[kc][:, 1 + s0 * (H + 1): 1 + (s0 + 4) * (H + 1)] \
                    .rearrange("p (s w) -> p s w", w=H + 1)[:, :, 0:H]
                nc.vector.tensor_copy(
                    dst, pp[:, :].rearrange("p (s w) -> p s w", w=H))
        for kc in range(4):
            _tap(nc, io, f"{tag}_g{g}slab{kc}", slabs[kc][:, :])
        return slabs

    def conv_grp(slabs, g, wts, bias_bc, acts, acts_dt):
        """3-tap conv for the 8 samples of group g; write biased acts."""
        for ls in range(SLAB):
            s = g * SLAB + ls
            cp = psum_conv.tile([128, 256], F32, tag="cp", name="cp")
            first = True
            for kc in range(4):
                for dh in range(3):
                    lhsT = slabs[kc][:, ls * (H + 1) + dh: ls * (H + 1) + dh + 128]
                    rhs = wts[kc][:, dh * 256:(dh + 1) * 256]
                    nc.tensor.matmul(cp[:, :], lhsT, rhs,
                                     start=first, stop=(kc == 3 and dh == 2))
                    first = False
            nc.vector.tensor_tensor(acts[:, s * 256:(s + 1) * 256],
                                    cp[:, :], bias_bc[:, :], ALU.add)
            if DEBUG_TAPS and s == 28 and acts.dtype != BF16:
                dbg = work.tile([128, 256], F32, tag="dbgcp", name="dbgcp")
                nc.vector.tensor_copy(dbg[:, :], cp[:, :])
                _tap(nc, io, "cp28", dbg[:, :])

    def big_linear(acts, w_dram, wdt, tag):
        """psum[j(128), b(32)] = sum_c W_c^T @ acts[:, (b, o=c)]."""
        lp = psum_lin.tile([128, BC], F32, tag="lp", name=f"{tag}_lp")
        for c in range(256):
            wsb = wpool.tile([128, 128], wdt, tag="w", name="w")
            nc.gpsimd.dma_start(wsb[:, :], w_dram[c * 128:(c + 1) * 128, :])
            rhs = acts[:, c:c + (BC - 1) * 256 + 1:256]
            nc.tensor.matmul(lp[:, :], wsb[:, :], rhs,
                             start=(c == 0), stop=(c == 255))
        return lp

    # ---------------- enemy branch ----------------
    xsb = wtile([BC, L], I32, "xsb")
    nc.gpsimd.dma_start(xsb[:, :], io["x"])
    xf = wtile([BC, L], F32, "xf")
    nc.vector.tensor_copy(xf[:, :], xsb[:, :])
    idxE = wtile([BC, H], F32, "idxE")
    nc.vector.scalar_tensor_tensor(idxE[:, :], xf[:, 0:L:2], float(V),
                                   xf[:, 1:L:2], ALU.mult, ALU.add)
    idxrowE = wtile([1, BC * H], F32, "idxrow")
    nc.gpsimd.dma_start(idxrowE[:, :], idxE[:, :])
    _tap(nc, io, "idxrowE", idxrowE[:, :])
    _tap(nc, io, "idxE", idxE[:, :])

    actsE = wtile([128, BC * 256], F32, "actsE")
    for g in range(NGRP):
        slabs = embed_pool_grp(idxrowE, g, tpE0, tpE1, F32R, "E")
        conv_grp(slabs, g, wtE, EBc, actsE, F32)

    _tap(nc, io, "actsE", actsE[:, :])
    lpE = big_linear(actsE, io["elw"], F32, "E")
    # softmax over j (partition dim): exp, sum via matmul, normalize
    Ex = wtile([128, BC], F32, "Ex")
    nc.scalar.activation(Ex[:, :], lpE[:, :], AF.Exp, bias=elb_col[:, :])
    s1 = psum_sm.tile([BC, 1], F32, tag="sm", name="s1")
    nc.tensor.matmul(s1[:, :], Ex[:, :], ones_col[:, :], start=True, stop=True)
    r32 = wtile([BC, 1], F32, "r32")
    nc.vector.reciprocal(r32[:, :], s1[:, :])
    rrow = wtile([1, BC], F32, "rrow")
    nc.gpsimd.dma_start(rrow[:, :], r32[:, :])
    rbp = psum_sm.tile([128, BC], F32, tag="sm", name="rbp")
    nc.tensor.matmul(rbp[:, :], ones_row[:, :], rrow[:, :], start=True, stop=True)
    _tap(nc, io, "Ex", Ex[:, :])
    vT = wtile([128, BC], F32R, "vT")   # enemy_out^T [i, b]
    nc.vector.tensor_tensor(vT[:, :], Ex[:, :], rbp[:, :], ALU.mult)

    # ---------------- manipulator ----------------
    rowsb = {}
    for name in ("int", "h0", "hL"):
        cx = psum_sm.tile([64, BC], F32, tag="sm", name="cx")
        nc.tensor.matmul(cx[:, :], wsumT[name][:, :],
                         vT[:, :], start=True, stop=True)
        cxs = work.tile([64, BC], F32, tag=f"cxs_{name}", name=f"cxs_{name}")
        nc.scalar.activation(cxs[:, :], cx[:, :], AF.Relu, bias=mcb_col[:, :])
        rowsb[name] = work.tile([1, 64 * BC], F32, tag="pm" if name == "int" else f"row_{name}",
                                name=f"row_{name}")
        nc.gpsimd.dma_start(rowsb[name][:, :], cxs[:, :])
    # assemble [128 h, (o, b)] manip acts: rows 1..126 = interior variant,
    # row 0 = h0 variant, row 127 = hL variant, via K=1 mask matmuls
    acts_m = wtile([128, 64 * BC], F32R, "acts_m")
    for nt in range(64 * BC // 512):
        amp = psum_emb.tile([128, 512], F32, tag="pp", name="amp")
        sl = slice(nt * 512, (nt + 1) * 512)
        nc.tensor.matmul(amp[:, :], ei_row[:, :], rowsb["int"][:, sl],
                         start=True, stop=False)
        nc.tensor.matmul(amp[:, :], e0_row[:, :], rowsb["h0"][:, sl],
                         start=False, stop=False)
        nc.tensor.matmul(amp[:, :], eL_row[:, :], rowsb["hL"][:, sl],
                         start=False, stop=True)
        nc.vector.tensor_copy(acts_m[:, sl], amp[:, :])

    mp = psum_lin.tile([BC, 256], F32, tag="lp", name="mp")
    for c in range(64):
        wsb = wpool.tile([128, 256], F32R, tag="w", name="w")
        nc.gpsimd.dma_start(wsb[:, :], io["mlw"][c * 128:(c + 1) * 128, :])
        nc.tensor.matmul(mp[:, :], acts_m[:, c * BC:(c + 1) * BC], wsb[:, :],
                         start=(c == 0), stop=(c == 63))
    m_sb = wtile([BC, 256], F32, "m_sb")
    nc.vector.tensor_tensor(m_sb[:, :], mp[:, :], MBc[0:BC, :], ALU.add)
    _tap(nc, io, "m", m_sb[:, :])

    # tokens = floor(|m|*100) mod 14; pair idx = 14*even + odd
    # floor via the 2^23 magic-number trick (t in [0, ~50) << 2^23):
    #   round_nearest(t - 0.5 + 2^23) - 2^23 == floor(t) for non-integer t
    # mod 14 via repeated conditional subtract (covers t < 42)
    tt = wtile([BC, 256], F32, "tt")
    nc.scalar.activation(tt[:, :], m_sb[:, :], AF.Abs, scale=100.0)
    fu = wtile([BC, 256], F32, "fu")
    nc.vector.tensor_scalar(fu[:, :], tt[:, :], 8388607.5, None, ALU.add)
    fr = wtile([BC, 256], F32, "fr")
    nc.vector.tensor_scalar(fr[:, :], fu[:, :], 8388608.0, None, ALU.subtract)
    ti = wtile([BC, 256], F32, "ti")
    nc.vector.tensor_scalar(ti[:, :], fr[:, :], float(V), None, ALU.is_ge)
    t1 = wtile([BC, 256], F32, "t1")
    nc.vector.scalar_tensor_tensor(t1[:, :], ti[:, :], -float(V), fr[:, :],
                                   ALU.mult, ALU.add)
    t2 = wtile([BC, 256], F32, "t2")
    nc.vector.tensor_scalar(t2[:, :], t1[:, :], float(V), None, ALU.is_ge)
    tok = wtile([BC, 256], F32, "tok")
    nc.vector.scalar_tensor_tensor(tok[:, :], t2[:, :], -float(V), t1[:, :],
                                   ALU.mult, ALU.add)
    _tap(nc, io, "tok", tok[:, :])
    idxF = wtile([BC, H], F32, "idxF")
    nc.vector.scalar_tensor_tensor(idxF[:, :], tok[:, 0:256:2], float(V),
                                   tok[:, 1:256:2], ALU.mult, ALU.add)
    idxrowF = wtile([1, BC * H], F32, "idxrow")
    nc.gpsimd.dma_start(idxrowF[:, :], idxF[:, :])

    # ---------------- friend branch (bf16) ----------------
    actsF = wtile([128, BC * 256], BF16, "actsF")
    for g in range(NGRP):
        slabs = embed_pool_grp(idxrowF, g, tpF0, tpF1, BF16, "F")
        conv_grp(slabs, g, wtF, FBc, actsF, BF16)

    _tap(nc, io, "actsF", actsF[:, :])
    _tap(nc, io, "vT", vT[:, :])
    lpF = big_linear(actsF, io["flw"], BF16, "F")
    fsb = wtile([128, BC], F32, "fsb")
    nc.vector.tensor_scalar(fsb[:, :], lpF[:, :], flb_col[:, :], None, ALU.add)

    w2sb = wtile([128, 14], F32, "w2sb")
    nc.gpsimd.dma_start(w2sb[:, :], io["f2w"])
    f2 = psum_sm.tile([BC, 14], F32, tag="sm", name="f2")
    nc.tensor.matmul(f2[:, :], fsb[:, :], w2sb[:, :], start=True, stop=True)
    logits = wtile([BC, 14], F32, "logits")
    nc.vector.tensor_tensor(logits[:, :], f2[:, :], F2Bc[0:BC, :], ALU.add)
    nmx = wtile([BC, 1], F32, "nmx")
    nc.vector.reduce_max(nmx[:, :], logits[:, :], AX.X, negate=True)
    ex = wtile([BC, 14], F32, "ex")
    nc.scalar.activation(ex[:, :], logits[:, :], AF.Exp, bias=nmx[:, :])
    sm = wtile([BC, 1], F32, "sm")
    nc.vector.reduce_sum(sm[:, :], ex[:, :], AX.X)
    rs = wtile([BC, 1], F32, "rs")
    nc.vector.reciprocal(rs[:, :], sm[:, :])
    outt = wtile([BC, 14], F32, "outt")
    nc.vector.tensor_scalar(outt[:, :], ex[:, :], rs[:, :], None, ALU.mult)
    nc.gpsimd.dma_start(io["out"], outt[:, :])


_CACHE = {}


def _get_nc():
    if "nc" not in _CACHE:
        nc = bacc.Bacc("TRN2", target_bir_lowering=False, debug=False,
                       num_devices=NCORES)
        with tile.TileContext(nc) as tc:
            with ExitStack() as ctx:
                build_kernel(nc, tc, ctx)
        nc.compile()
        _CACHE["nc"] = nc
    return _CACHE["nc"]


def prep_inputs(inputs):
    """Host-side shard/layout prep. Returns list of 8 in_maps."""
    f32 = np.float32
    bf16 = ml_dtypes.bfloat16
    common = {
        "eemb": np.ascontiguousarray(inputs["enemy_emb"], f32),
        "ecw": np.ascontiguousarray(
            np.asarray(inputs["enemy_conv_w"])[:, :, :, 1], f32).reshape(256, -1),
        "ecb": np.ascontiguousarray(inputs["enemy_conv_b"], f32),
        "elw": np.ascontiguousarray(inputs["enemy_lin_w"], f32),
        "elb": np.ascontiguousarray(inputs["enemy_lin_b"], f32),
        "mcw": np.ascontiguousarray(
            np.asarray(inputs["manip_conv_w"])[:, :, :, 1], f32).reshape(64, -1),
        "mcb": np.ascontiguousarray(inputs["manip_conv_b"], f32),
        "mlw": np.ascontiguousarray(inputs["manip_lin_w"], f32),
        "mlb": np.ascontiguousarray(inputs["manip_lin_b"], f32),
        "femb": np.asarray(inputs["friend_emb"]).astype(bf16),
        "fcw": np.ascontiguousarray(
            np.asarray(inputs["friend_conv_w"])[:, :, :, 1]).reshape(256, -1)
            .astype(bf16),
        "fcb": np.ascontiguousarray(inputs["friend_conv_b"], f32),
        "flw": np.asarray(inputs["friend_lin1_w"]).astype(bf16),
        "flb": np.ascontiguousarray(inputs["friend_lin1_b"], f32),
        "f2w": np.ascontiguousarray(inputs["friend_lin2_w"], f32),
        "f2b": np.ascontiguousarray(inputs["friend_lin2_b"], f32),
    }
    x = np.ascontiguousarray(inputs["x"], np.int32)
    return [dict(common, x=np.ascontiguousarray(x[c * BC:(c + 1) * BC]))
            for c in range(NCORES)]


def kernel(**inputs):
    nc = _get_nc()
    in_maps = prep_inputs(inputs)
    res = run_bass_kernel_spmd(nc, in_maps, core_ids=list(range(NCORES)))
    return np.concatenate([r["out"] for r in res.results], axis=0)



# revision 2
# speedup vs baseline: 4.6207x; 4.6207x over previous
"""Trainium2 Bass kernel for nn_Network_67388036874689.

Data-parallel over batch: B=256 sharded as 32 samples on each of 8 cores;
all parameters replicated (host-precomposed).

Structure exploited (validated against the reference on host):
  - fog_of_war's greedy scan returns arange(B) -> the permutation is identity.
  - conv2d(3x3, pad=1) on [C, H, 1] spatial input only sees kernel column 1
    -> 1D conv over H with 3 taps.
  - Embedding (V=14) + pair-maxpool + conv compose into per-tap tables
    CW[kh] = pairmax_table @ conv_w[:, :, kh].T  (196 x 256), host-built.
    Device conv = one-hot(pair idx) matmuls against CW with +-1 shifts.
  - Conv bias folds into the following linear's bias (host).
  - The manipulator conv input is constant over h -> the 8192x256 manip
    linear collapses to 3 reduced 64x256 matrices (host-summed over h).

Precision: tables/linears in bf16 (host sim: 2/65536 token flips,
rel err ~1e-4); manipulator path f32/f32r; all psum accumulation f32.
"""

import numpy as np
import ml_dtypes
from contextlib import ExitStack

import concourse.bass as bass
import concourse.bacc as bacc
import concourse.mybir as mybir
import concourse.tile as tile
from concourse.masks import make_identity
from concourse.bass_utils import run_bass_kernel_spmd

F32 = mybir.dt.float32
F32R = mybir.dt.float32r
BF16 = mybir.dt.bfloat16
I32 = mybir.dt.int32
AF = mybir.ActivationFunctionType
ALU = mybir.AluOpType
AX = mybir.AxisListType

NCORES = 8
B = 256
BC = B // NCORES        # 32 samples per core
L = 256
V = 14
EMB = 512
H = L // 2              # 128 pooled positions
NPAIR = V * V           # 196
P0 = 112                # pair-table partition split: 112 + 84
P1 = NPAIR - P0
SW = H + 2              # 130: per-sample padded width in the one-hot tiles
OHW = BC * SW           # 4160
DEBUG_TAPS = False


def _dram_inputs(nc):
    t = {}

    def inp(name, shape, dt):
        t[name] = nc.dram_tensor(name, list(shape), dt, kind="ExternalInput").ap()

    inp("x", (BC, L), I32)
    inp("cwE0", (P0, 768), BF16)     # enemy CW tables, col = kh*256 + o
    inp("cwE1", (P1, 768), BF16)
    inp("cwF0", (P0, 768), BF16)
    inp("cwF1", (P1, 768), BF16)
    inp("elw3", (256, 128 * 128), BF16)   # [o, (h, j)]
    inp("flw3", (256, 128 * 128), BF16)
    inp("mlwS", (64, 768), F32R)     # col = v*256 + j, v in (int, h0, hL)
    inp("wsumT", (128, 192), F32R)   # col = v*64 + o
    inp("mcb", (64,), F32)
    inp("elbe", (128,), F32)         # enemy lin bias + folded conv bias
    inp("flbe", (128,), F32)
    inp("mlb", (256,), F32)
    inp("f2w", (128, 14), F32)
    inp("f2b", (14,), F32)
    t["out"] = nc.dram_tensor("out", [BC, 14], F32, kind="ExternalOutput").ap()
    return t


def _tap(nc, io, name, ap):
    if not DEBUG_TAPS:
        return
    t = nc.dram_tensor("tap_" + name, list(ap.shape), ap.dtype,
                       kind="ExternalOutput").ap()
    io["tap_" + name] = t
    nc.gpsimd.dma_start(t, ap)


def build_kernel(nc, tc, ctx):
    io = _dram_inputs(nc)
    consts = ctx.enter_context(tc.tile_pool(name="consts", bufs=1))
    work = ctx.enter_context(tc.tile_pool(name="work", bufs=1))
    wpool = ctx.enter_context(tc.tile_pool(name="wstream", bufs=4))
    ohpool = ctx.enter_context(tc.tile_pool(name="ohpool", bufs=1))
    ppp = ctx.enter_context(tc.tile_pool(name="ppp", bufs=2, space="PSUM"))
    pconv = ctx.enter_context(tc.tile_pool(name="pconv", bufs=4, space="PSUM"))
    plin = ctx.enter_context(tc.tile_pool(name="plin", bufs=1, space="PSUM"))
    psm = ctx.enter_context(tc.tile_pool(name="psm", bufs=1, space="PSUM"))

    def ctile(shape, dt, tag):
        return consts.tile(shape, dt, tag=tag, name=tag)

    def wtile(shape, dt, tag):
        return work.tile(shape, dt, tag=tag, name=tag)

    # ---------------- constants & small weights ----------------
    identF = ctile([128, 128], F32, "identF")
    make_identity(nc, identF)
    iota_i = ctile([128, 1], I32, "iota_i")
    nc.gpsimd.iota(iota_i[:, :], pattern=[[0, 1]], base=0, channel_multiplier=1)
    iota_col = ctile([128, 1], F32, "iota_col")
    nc.vector.tensor_copy(iota_col[:, :], iota_i[:, :])
    ones_row = ctile([1, 128], BF16, "ones_row")
    nc.vector.memset(ones_row[:, :], 1.0)

    def bias_col(dram_vec, n, tag):
        col = ctile([n, 1], F32, tag)
        nc.gpsimd.dma_start(col[:, :], dram_vec)
        return col

    def bias_bcast(dram_vec, rows, width, tag):
        out = ctile([rows, width], F32, tag)
        nc.gpsimd.dma_start(out[:, :], dram_vec[None, :].partition_broadcast(rows))
        return out

    elbeB = bias_bcast(io["elbe"], BC, 128, "elbeB")
    flbeB = bias_bcast(io["flbe"], BC, 128, "flbeB")
    mlbB = bias_bcast(io["mlb"], BC, 256, "mlbB")
    f2bB = bias_bcast(io["f2b"], BC, 14, "f2bB")
    mcb_col = bias_col(io["mcb"], 64, "mcb")

    def load(name, shape, dt):
        t = ctile(shape, dt, name)
        nc.sync.dma_start(t[:, :], io[name])
        return t

    cwE0 = load("cwE0", [P0, 768], BF16)
    cwE1 = load("cwE1", [P1, 768], BF16)
    cwF0 = load("cwF0", [P0, 768], BF16)
    cwF1 = load("cwF1", [P1, 768], BF16)
    mlwS = load("mlwS", [64, 768], F32R)
    wsumT = load("wsumT", [128, 192], F32R)
    w2sb = load("f2w", [128, 14], F32)

    # ---------------- streamed big weights ----------------
    # elw3/flw3: [o(256), (h,j)]; piece = [o-half(128), 64 h x 128 j] = 2 MB
    def stream_weights(dram):
        pieces = []
        for half in range(2):
            for hb in range(2):
                p = wpool.tile([128, 64 * 128], BF16, tag="wp", name="wp")
                nc.sync.dma_start(
                    p[:, :], dram[half * 128:(half + 1) * 128,
                                  hb * 8192:(hb + 1) * 8192])
                pieces.append(p)
        return pieces

    elwP = stream_weights(io["elw3"])
    flwP = stream_weights(io["flw3"])

    # ---------------- stage helpers ----------------
    def build_oh(idxrow, tag):
        """One-hot over pair idx for all 32 samples, padded layout:
        col s*130 + 1 + h holds [idx[s,h] == t]; cols s*130 and s*130+129
        are zero (conv boundary)."""
        oh0 = ohpool.tile([P0, OHW], BF16, tag="oh0", name=f"oh0{tag}")
        oh1 = ohpool.tile([P1, OHW], BF16, tag="oh1", name=f"oh1{tag}")
        for oh in (oh0, oh1):
            nc.vector.memset(oh[:, 0:OHW:SW], 0.0)
            nc.vector.memset(oh[:, SW - 1:OHW:SW], 0.0)
        for blk in range(8):
            pp = ppp.tile([P0, 512], F32, tag="pp", name="pp")
            nc.tensor.matmul(pp[:, :], ones_row[:, 0:P0],
                             idxrow[:, blk * 512:(blk + 1) * 512],
                             start=True, stop=True)
            src = pp[:, :].rearrange("p (s w) -> p s w", w=128)
            dst0 = oh0[:, blk * 4 * SW:(blk + 1) * 4 * SW] \
                .rearrange("p (s w) -> p s w", w=SW)[:, :, 1:129]
            nc.vector.tensor_scalar(dst0, src, iota_col[0:P0, :], None,
                                    ALU.is_equal)
            dst1 = oh1[:, blk * 4 * SW:(blk + 1) * 4 * SW] \
                .rearrange("p (s w) -> p s w", w=SW)[:, :, 1:129]
            nc.vector.tensor_scalar(dst1, src[0:P1], float(P0),
                                    iota_col[0:P1, :], ALU.subtract,
                                    ALU.is_equal)
        return oh0, oh1

    def conv_apply(oh0, oh1, cw0, cw1, tag):
        """y[o, (s,h)] = sum_kh CW_kh[idx[h+kh-1], o]; acts as 2 halves
        [128 o', 32*128 (s,h)] bf16."""
        acts = [wtile([128, BC * H], BF16, f"acts{tag}{oc}") for oc in range(2)]
        for oc in range(2):
            for blk in range(8):
                cp = pconv.tile([128, 512], F32, tag="cp", name="cp")
                n = 0
                for cw, oh, npart in ((cw0, oh0, P0), (cw1, oh1, P1)):
                    for kh in range(3):
                        lhsT = cw[:, kh * 256 + oc * 128:
                                  kh * 256 + (oc + 1) * 128]
                        rhs = oh[:, blk * 4 * SW:(blk + 1) * 4 * SW] \
                            .rearrange("p (s w) -> p s w", w=SW)[:, :, kh:kh + 128]
                        nc.tensor.matmul(cp[:, :], lhsT, rhs,
                                         start=(n == 0), stop=(n == 5))
                        n += 1
                nc.vector.tensor_copy(
                    acts[oc][:, blk * 512:(blk + 1) * 512], cp[:, :])
        return acts

    def big_linear(acts, pieces, tag):
        """lp[s, j] = sum_{o,h} acts[o][:, s*128+h] * W[(o,h), j]"""
        lp = plin.tile([BC, 128], F32, tag="lp", name=f"lp{tag}")
        for half in range(2):
            for h in range(128):
                piece = pieces[half * 2 + h // 64]
                lhsT = acts[half][:, h:h + (BC - 1) * 128 + 1:128]
                rhs = piece[:, (h % 64) * 128:(h % 64 + 1) * 128]
                nc.tensor.matmul(lp[:, :], lhsT, rhs,
                                 start=(half == 0 and h == 0),
                                 stop=(half == 1 and h == 127))
        return lp

    # ---------------- enemy branch ----------------
    xsb = wtile([BC, L], I32, "xsb")
    nc.sync.dma_start(xsb[:, :], io["x"])
    xf = wtile([BC, L], F32, "xf")
    nc.vector.tensor_copy(xf[:, :], xsb[:, :])
    idxE = wtile([BC, H], BF16, "idxE")
    nc.vector.scalar_tensor_tensor(idxE[:, :], xf[:, 0:L:2], float(V),
                                   xf[:, 1:L:2], ALU.mult, ALU.add)
    idxrowE = wtile([1, BC * H], BF16, "idxrowE")
    nc.gpsimd.dma_start(idxrowE[:, :], idxE[:, :])

    ohE0, ohE1 = build_oh(idxrowE, "E")
    actsE = conv_apply(ohE0, ohE1, cwE0, cwE1, "E")
    _tap(nc, io, "actsE0", actsE[0][:, :])
    lpE = big_linear(actsE, elwP, "E")

    logitsE = wtile([BC, 128], F32, "logitsE")
    nc.vector.tensor_tensor(logitsE[:, :], lpE[:, :], elbeB[:, :], ALU.add)
    _tap(nc, io, "logitsE", logitsE[:, :])
    nmxE = wtile([BC, 1], F32, "nmxE")
    nc.vector.reduce_max(nmxE[:, :], logitsE[:, :], AX.X, negate=True)
    ExE = wtile([BC, 128], F32, "ExE")
    nc.scalar.activation(ExE[:, :], logitsE[:, :], AF.Exp, bias=nmxE[:, :])
    smE = wtile([BC, 1], F32, "smE")
    nc.vector.reduce_sum(smE[:, :], ExE[:, :], AX.X)
    rsE = wtile([BC, 1], F32, "rsE")
    nc.vector.reciprocal(rsE[:, :], smE[:, :])
    eout = wtile([BC, 128], F32, "eout")
    nc.vector.tensor_scalar(eout[:, :], ExE[:, :], rsE[:, :], None, ALU.mult)

    tpv = psm.tile([128, BC], F32, tag="sm", name="tpv")
    nc.tensor.transpose(tpv[:, :], eout[:, :], identF[0:BC, 0:BC])
    vT = wtile([128, BC], F32R, "vT")
    nc.vector.tensor_copy(vT[:, :], tpv[:, :])
    _tap(nc, io, "vT", vT[:, :])

    # ---------------- manipulator ----------------
    cxs = {}
    for i, v in enumerate(("int", "h0", "hL")):
        cx = psm.tile([64, BC], F32, tag="sm", name=f"cx{v}")
        nc.tensor.matmul(cx[:, :], wsumT[:, i * 64:(i + 1) * 64], vT[:, :],
                         start=True, stop=True)
        cxs[v] = wtile([64, BC], F32R, f"cxs_{v}")
        nc.scalar.activation(cxs[v][:, :], cx[:, :], AF.Relu, bias=mcb_col[:, :])
    mp = plin.tile([BC, 256], F32, tag="lp", name="mp")
    for i, v in enumerate(("int", "h0", "hL")):
        nc.tensor.matmul(mp[:, :], cxs[v][:, :], mlwS[:, i * 256:(i + 1) * 256],
                         start=(i == 0), stop=(i == 2))
    m_sb = wtile([BC, 256], F32, "m_sb")
    nc.vector.tensor_tensor(m_sb[:, :], mp[:, :], mlbB[:, :], ALU.add)
    _tap(nc, io, "m", m_sb[:, :])

    # tokens = floor(|m|*100) mod 14; pair idx = 14*even + odd
    # floor via the 2^23 magic-number trick; mod 14 via 2 conditional subtracts
    tt = wtile([BC, 256], F32, "tt")
    nc.scalar.activation(tt[:, :], m_sb[:, :], AF.Abs, scale=100.0)
    fu = wtile([BC, 256], F32, "fu")
    nc.vector.tensor_scalar(fu[:, :], tt[:, :], 8388607.5, None, ALU.add)
    fr = wtile([BC, 256], F32, "fr")
    nc.vector.tensor_scalar(fr[:, :], fu[:, :], 8388608.0, None, ALU.subtract)
    ti = wtile([BC, 256], F32, "ti")
    nc.vector.tensor_scalar(ti[:, :], fr[:, :], float(V), None, ALU.is_ge)
    t1 = wtile([BC, 256], F32, "t1")
    nc.vector.scalar_tensor_tensor(t1[:, :], ti[:, :], -float(V), fr[:, :],
                                   ALU.mult, ALU.add)
    t2 = wtile([BC, 256], F32, "t2")
    nc.vector.tensor_scalar(t2[:, :], t1[:, :], float(V), None, ALU.is_ge)
    tok = wtile([BC, 256], F32, "tok")
    nc.vector.scalar_tensor_tensor(tok[:, :], t2[:, :], -float(V), t1[:, :],
                                   ALU.mult, ALU.add)
    _tap(nc, io, "tok", tok[:, :])
    idxF = wtile([BC, H], BF16, "idxF")
    nc.vector.scalar_tensor_tensor(idxF[:, :], tok[:, 0:256:2], float(V),
                                   tok[:, 1:256:2], ALU.mult, ALU.add)
    idxrowF = wtile([1, BC * H], BF16, "idxrowF")
    nc.gpsimd.dma_start(idxrowF[:, :], idxF[:, :])

    # ---------------- friend branch ----------------
    ohF0, ohF1 = build_oh(idxrowF, "F")
    actsF = conv_apply(ohF0, ohF1, cwF0, cwF1, "F")
    lpF = big_linear(actsF, flwP, "F")
    fsb = wtile([BC, 128], F32, "fsb")
    nc.vector.tensor_tensor(fsb[:, :], lpF[:, :], flbeB[:, :], ALU.add)

    tpf = psm.tile([128, BC], F32, tag="sm", name="tpf")
    nc.tensor.transpose(tpf[:, :], fsb[:, :], identF[0:BC, 0:BC])
    fT = wtile([128, BC], F32, "fT")
    nc.vector.tensor_copy(fT[:, :], tpf[:, :])
    f2 = psm.tile([BC, 14], F32, tag="sm", name="f2")
    nc.tensor.matmul(f2[:, :], fT[:, :], w2sb[:, :], start=True, stop=True)
    logits = wtile([BC, 14], F32, "logits")
    nc.vector.tensor_tensor(logits[:, :], f2[:, :], f2bB[:, :], ALU.add)
    nmx = wtile([BC, 1], F32, "nmx")
    nc.vector.reduce_max(nmx[:, :], logits[:, :], AX.X, negate=True)
    ex = wtile([BC, 14], F32, "ex")
    nc.scalar.activation(ex[:, :], logits[:, :], AF.Exp, bias=nmx[:, :])
    sm = wtile([BC, 1], F32, "sm")
    nc.vector.reduce_sum(sm[:, :], ex[:, :], AX.X)
    rs = wtile([BC, 1], F32, "rs")
    nc.vector.reciprocal(rs[:, :], sm[:, :])
    outt = wtile([BC, 14], F32, "outt")
    nc.vector.tensor_scalar(outt[:, :], ex[:, :], rs[:, :], None, ALU.mult)
    nc.sync.dma_start(io["out"], outt[:, :])


_CACHE = {}


def _get_nc():
    if "nc" not in _CACHE:
        nc = bacc.Bacc("TRN2", target_bir_lowering=False, debug=False,
                       num_devices=NCORES)
        with tile.TileContext(nc) as tc:
            with ExitStack() as ctx:
                build_kernel(nc, tc, ctx)
        nc.compile()
        _CACHE["nc"] = nc
    return _CACHE["nc"]


def prep_inputs(inputs):
    """Host-side composition + shard. Returns list of 8 in_maps."""
    f32 = np.float32
    bf16 = ml_dtypes.bfloat16

    def cw_tables(emb, cw_full):
        emb = np.asarray(emb, f32)
        cw = np.ascontiguousarray(np.asarray(cw_full, f32)[:, :, :, 1])  # [O,I,3]
        t0, t1 = np.meshgrid(np.arange(V), np.arange(V), indexing="ij")
        table = np.maximum(emb[t0.ravel()], emb[t1.ravel()])            # [196,512]
        cwc = np.concatenate([table @ cw[:, :, kh].T for kh in range(3)],
                             axis=1).astype(bf16)                        # [196,768]
        return np.ascontiguousarray(cwc[:P0]), np.ascontiguousarray(cwc[P0:])

    cwE0, cwE1 = cw_tables(inputs["enemy_emb"], inputs["enemy_conv_w"])
    cwF0, cwF1 = cw_tables(inputs["friend_emb"], inputs["friend_conv_w"])

    elw = np.asarray(inputs["enemy_lin_w"], f32)
    flw = np.asarray(inputs["friend_lin1_w"], f32)
    elbe = (np.asarray(inputs["enemy_lin_b"], f32)
            + np.einsum("o,ohj->j", np.asarray(inputs["enemy_conv_b"], f32),
                        elw.reshape(256, 128, 128), optimize=True)).astype(f32)
    flbe = (np.asarray(inputs["friend_lin1_b"], f32)
            + np.einsum("o,ohj->j", np.asarray(inputs["friend_conv_b"], f32),
                        flw.reshape(256, 128, 128), optimize=True)).astype(f32)

    mcw = np.asarray(inputs["manip_conv_w"], f32)[:, :, :, 1]  # [64,128,3]
    s_int = mcw.sum(2)
    s12 = mcw[:, :, 1] + mcw[:, :, 2]
    s01 = mcw[:, :, 0] + mcw[:, :, 1]
    wsumT = np.concatenate([s_int.T, s12.T, s01.T], axis=1).astype(f32)  # [128,192]

    mlw3 = np.asarray(inputs["manip_lin_w"], f32).reshape(64, 128, 256)
    mlwS = np.concatenate([mlw3[:, 1:127].sum(1), mlw3[:, 0], mlw3[:, 127]],
                          axis=1).astype(f32)                            # [64,768]

    common = {
        "cwE0": cwE0, "cwE1": cwE1, "cwF0": cwF0, "cwF1": cwF1,
        "elw3": np.ascontiguousarray(elw.reshape(256, 128 * 128)).astype(bf16),
        "flw3": np.ascontiguousarray(flw.reshape(256, 128 * 128)).astype(bf16),
        "mlwS": np.ascontiguousarray(mlwS),
        "wsumT": np.ascontiguousarray(wsumT),
        "mcb": np.ascontiguousarray(inputs["manip_conv_b"], f32),
        "elbe": elbe,
        "flbe": flbe,
        "mlb": np.ascontiguousarray(inputs["manip_lin_b"], f32),
        "f2w": np.ascontiguousarray(inputs["friend_lin2_w"], f32),
        "f2b": np.ascontiguousarray(inputs["friend_lin2_b"], f32),
    }
    x = np.ascontiguousarray(np.asarray(inputs["x"], np.int32))
    return [dict(common, x=np.ascontiguousarray(x[c * BC:(c + 1) * BC]))
            for c in range(NCORES)]


def kernel(**inputs):
    nc = _get_nc()
    in_maps = prep_inputs(inputs)
    res = run_bass_kernel_spmd(nc, in_maps, core_ids=list(range(NCORES)))
    return np.concatenate([r["out"] for r in res.results], axis=0)


# revision 5
# speedup vs baseline: 4.9714x; 1.0759x over previous
"""Trainium2 Bass kernel for nn_Network_67388036874689.

Data-parallel over batch: B=256 sharded as 32 samples on each of 8 cores;
all parameters replicated (host-precomposed).

Structure exploited (validated against the reference on host):
  - fog_of_war's greedy scan returns arange(B) -> the permutation is identity.
  - conv2d(3x3, pad=1) on [C, H, 1] spatial input only sees kernel column 1
    -> 1D conv over H with 3 taps.
  - Embedding (V=14) + pair-maxpool + conv compose into per-tap tables
    CW[kh] = pairmax_table @ conv_w[:, :, kh].T  (196 x 256), host-built.
    Device conv = one-hot(pair idx) matmuls against CW with +-1 shifts.
  - Conv bias folds into the following linear's bias (host).
  - The manipulator conv input is constant over h -> the 8192x256 manip
    linear collapses to 3 reduced 64x256 matrices (host-summed over h).

Precision: tables/linears in bf16 (host sim: 2/65536 token flips,
rel err ~1e-4); manipulator path f32/f32r; all psum accumulation f32.
"""

import numpy as np
import ml_dtypes
from contextlib import ExitStack

import concourse.bass as bass
import concourse.bacc as bacc
import concourse.mybir as mybir
import concourse.tile as tile
from concourse.masks import make_identity
from concourse.bass_utils import run_bass_kernel_spmd

F32 = mybir.dt.float32
F32R = mybir.dt.float32r
BF16 = mybir.dt.bfloat16
I32 = mybir.dt.int32
AF = mybir.ActivationFunctionType
ALU = mybir.AluOpType
AX = mybir.AxisListType

NCORES = 8
B = 256
BC = B // NCORES        # 32 samples per core
L = 256
V = 14
EMB = 512
H = L // 2              # 128 pooled positions
NPAIR = V * V           # 196
P0 = 112                # pair-table partition split: 112 + 84
P1 = NPAIR - P0
SW = H + 2              # 130: per-sample padded width in the one-hot tiles
OHW = BC * SW           # 4160
DEBUG_TAPS = False


def _dram_inputs(nc):
    t = {}

    def inp(name, shape, dt):
        t[name] = nc.dram_tensor(name, list(shape), dt, kind="ExternalInput").ap()

    inp("x", (BC, L), I32)
    inp("cwE0", (P0, 768), BF16)     # enemy CW tables, col = kh*256 + o
    inp("cwE1", (P1, 768), BF16)
    inp("cwF0", (P0, 768), BF16)
    inp("cwF1", (P1, 768), BF16)
    inp("elw3", (256, 128 * 128), BF16)   # [o, (h, j)]
    inp("flw3", (256, 128 * 128), BF16)
    inp("mlwS", (64, 768), F32R)     # col = v*256 + j, v in (int, h0, hL)
    inp("wsumT", (128, 192), F32R)   # col = v*64 + o
    inp("mcb", (64,), F32)
    inp("elbe", (128,), F32)         # enemy lin bias + folded conv bias
    inp("flbe", (128,), F32)
    inp("mlb", (256,), F32)
    inp("f2w", (128, 14), F32)
    inp("f2b", (14,), F32)
    t["out"] = nc.dram_tensor("out", [BC, 14], F32, kind="ExternalOutput").ap()
    return t


def _tap(nc, io, name, ap):
    if not DEBUG_TAPS:
        return
    t = nc.dram_tensor("tap_" + name, list(ap.shape), ap.dtype,
                       kind="ExternalOutput").ap()
    io["tap_" + name] = t
    nc.gpsimd.dma_start(t, ap)


def build_kernel(nc, tc, ctx):
    io = _dram_inputs(nc)
    consts = ctx.enter_context(tc.tile_pool(name="consts", bufs=1))
    work = ctx.enter_context(tc.tile_pool(name="work", bufs=1))
    wpool = ctx.enter_context(tc.tile_pool(name="wstream", bufs=4))
    ohpool = ctx.enter_context(tc.tile_pool(name="ohpool", bufs=1))
    ppp = ctx.enter_context(tc.tile_pool(name="ppp", bufs=2, space="PSUM"))
    pconv = ctx.enter_context(tc.tile_pool(name="pconv", bufs=4, space="PSUM"))
    plin = ctx.enter_context(tc.tile_pool(name="plin", bufs=1, space="PSUM"))
    psm = ctx.enter_context(tc.tile_pool(name="psm", bufs=1, space="PSUM"))

    def ctile(shape, dt, tag):
        return consts.tile(shape, dt, tag=tag, name=tag)

    def wtile(shape, dt, tag):
        return work.tile(shape, dt, tag=tag, name=tag)

    # ---------------- constants & small weights ----------------
    identF = ctile([128, 128], F32, "identF")
    make_identity(nc, identF)
    iota_i = ctile([128, 1], I32, "iota_i")
    nc.gpsimd.iota(iota_i[:, :], pattern=[[0, 1]], base=0, channel_multiplier=1)
    iota_col = ctile([128, 1], F32, "iota_col")
    nc.vector.tensor_copy(iota_col[:, :], iota_i[:, :])
    ones_row = ctile([1, 128], BF16, "ones_row")
    nc.vector.memset(ones_row[:, :], 1.0)

    def bias_col(dram_vec, n, tag):
        col = ctile([n, 1], F32, tag)
        nc.gpsimd.dma_start(col[:, :], dram_vec)
        return col

    def bias_bcast(dram_vec, rows, width, tag):
        out = ctile([rows, width], F32, tag)
        nc.gpsimd.dma_start(out[:, :], dram_vec[None, :].partition_broadcast(rows))
        return out

    # x first on the sync HWDGE ring: the whole front of the kernel needs it
    xsb = wtile([BC, L], I32, "xsb")
    nc.sync.dma_start(xsb[:, :], io["x"])

    elbeB = bias_bcast(io["elbe"], BC, 128, "elbeB")
    flbeB = bias_bcast(io["flbe"], BC, 128, "flbeB")
    mlbB = bias_bcast(io["mlb"], BC, 256, "mlbB")
    f2bB = bias_bcast(io["f2b"], BC, 14, "f2bB")
    mcb_col = bias_col(io["mcb"], 64, "mcb")

    def load(name, shape, dt):
        t = ctile(shape, dt, name)
        nc.sync.dma_start(t[:, :], io[name])
        return t

    cwE0 = load("cwE0", [P0, 768], BF16)
    cwE1 = load("cwE1", [P1, 768], BF16)
    cwF0 = load("cwF0", [P0, 768], BF16)
    cwF1 = load("cwF1", [P1, 768], BF16)
    mlwS = load("mlwS", [64, 768], F32R)
    wsumT = load("wsumT", [128, 192], F32R)
    w2sb = load("f2w", [128, 14], F32)

    # ---------------- streamed big weights ----------------
    # elw3/flw3: [o(256), (h,j)]; piece = [o-half(128), 64 h x 128 j] = 2 MB.
    # On the scalar-engine HWDGE ring so they don't queue behind (or ahead
    # of) the small sync-ring loads.
    def stream_weights(dram):
        pieces = []
        for half in range(2):
            for hb in range(2):
                p = wpool.tile([128, 64 * 128], BF16, tag="wp", name="wp")
                nc.scalar.dma_start(
                    p[:, :], dram[half * 128:(half + 1) * 128,
                                  hb * 8192:(hb + 1) * 8192])
                pieces.append(p)
        return pieces

    elwP = stream_weights(io["elw3"])
    flwP = stream_weights(io["flw3"])

    # ---------------- stage helpers ----------------
    def build_oh(idxrow, tag):
        """One-hot over pair idx for all 32 samples, padded layout:
        col s*130 + 1 + h holds [idx[s,h] == t]; cols s*130 and s*130+129
        are zero (conv boundary)."""
        oh0 = ohpool.tile([P0, OHW], BF16, tag="oh0", name=f"oh0{tag}")
        oh1 = ohpool.tile([P1, OHW], BF16, tag="oh1", name=f"oh1{tag}")
        for oh in (oh0, oh1):
            nc.vector.memset(oh[:, 0:OHW:SW], 0.0)
            nc.vector.memset(oh[:, SW - 1:OHW:SW], 0.0)
        for blk in range(8):
            pp = ppp.tile([P0, 512], F32, tag="pp", name="pp")
            nc.tensor.matmul(pp[:, :], ones_row[:, 0:P0],
                             idxrow[:, blk * 512:(blk + 1) * 512],
                             start=True, stop=True)
            src = pp[:, :].rearrange("p (s w) -> p s w", w=128)
            dst0 = oh0[:, blk * 4 * SW:(blk + 1) * 4 * SW] \
                .rearrange("p (s w) -> p s w", w=SW)[:, :, 1:129]
            nc.vector.tensor_scalar(dst0, src, iota_col[0:P0, :], None,
                                    ALU.is_equal)
            dst1 = oh1[:, blk * 4 * SW:(blk + 1) * 4 * SW] \
                .rearrange("p (s w) -> p s w", w=SW)[:, :, 1:129]
            nc.vector.tensor_scalar(dst1, src[0:P1], float(P0),
                                    iota_col[0:P1, :], ALU.subtract,
                                    ALU.is_equal)
        return oh0, oh1

    def conv_apply(oh0, oh1, cw0, cw1, tag):
        """y[o, (s,h)] = sum_kh CW_kh[idx[h+kh-1], o]; acts as 2 halves
        [128 o', 32*128 (s,h)] bf16."""
        acts = [wtile([128, BC * H], BF16, f"acts{tag}{oc}") for oc in range(2)]
        for oc in range(2):
            for blk in range(8):
                cp = pconv.tile([128, 512], F32, tag="cp", name="cp")
                n = 0
                for cw, oh, npart in ((cw0, oh0, P0), (cw1, oh1, P1)):
                    for kh in range(3):
                        lhsT = cw[:, kh * 256 + oc * 128:
                                  kh * 256 + (oc + 1) * 128]
                        rhs = oh[:, blk * 4 * SW:(blk + 1) * 4 * SW] \
                            .rearrange("p (s w) -> p s w", w=SW)[:, :, kh:kh + 128]
                        nc.tensor.matmul(cp[:, :], lhsT, rhs,
                                         start=(n == 0), stop=(n == 5))
                        n += 1
                dst = acts[oc][:, blk * 512:(blk + 1) * 512]
                if blk % 2 == 0:
                    nc.scalar.activation(dst, cp[:, :], AF.Copy)
                else:
                    nc.vector.tensor_copy(dst, cp[:, :])
        return acts

    def big_linear(acts, pieces, tag):
        """lp[s, j] = sum_{o,h} acts[o][:, s*128+h] * W[(o,h), j]"""
        lp = plin.tile([BC, 128], F32, tag="lp", name=f"lp{tag}")
        for half in range(2):
            for h in range(128):
                piece = pieces[half * 2 + h // 64]
                lhsT = acts[half][:, h:h + (BC - 1) * 128 + 1:128]
                rhs = piece[:, (h % 64) * 128:(h % 64 + 1) * 128]
                nc.tensor.matmul(lp[:, :], lhsT, rhs,
                                 start=(half == 0 and h == 0),
                                 stop=(half == 1 and h == 127))
        return lp

    # ---------------- enemy branch ----------------
    xf = wtile([BC, L], F32, "xf")
    nc.vector.tensor_copy(xf[:, :], xsb[:, :])
    idxE = wtile([BC, H], BF16, "idxE")
    nc.vector.scalar_tensor_tensor(idxE[:, :], xf[:, 0:L:2], float(V),
                                   xf[:, 1:L:2], ALU.mult, ALU.add)
    idxrowE = wtile([1, BC * H], BF16, "idxrowE")
    nc.gpsimd.dma_start(idxrowE[:, :], idxE[:, :])

    ohE0, ohE1 = build_oh(idxrowE, "E")
    actsE = conv_apply(ohE0, ohE1, cwE0, cwE1, "E")
    _tap(nc, io, "actsE0", actsE[0][:, :])
    lpE = big_linear(actsE, elwP, "E")

    logitsE = wtile([BC, 128], F32, "logitsE")
    nc.vector.tensor_tensor(logitsE[:, :], lpE[:, :], elbeB[:, :], ALU.add)
    _tap(nc, io, "logitsE", logitsE[:, :])
    nmxE = wtile([BC, 1], F32, "nmxE")
    nc.vector.reduce_max(nmxE[:, :], logitsE[:, :], AX.X, negate=True)
    ExE = wtile([BC, 128], F32, "ExE")
    nc.scalar.activation(ExE[:, :], logitsE[:, :], AF.Exp, bias=nmxE[:, :])
    smE = wtile([BC, 1], F32, "smE")
    nc.vector.reduce_sum(smE[:, :], ExE[:, :], AX.X)
    rsE = wtile([BC, 1], F32, "rsE")
    nc.vector.reciprocal(rsE[:, :], smE[:, :])
    eout = wtile([BC, 128], F32, "eout")
    nc.vector.tensor_scalar(eout[:, :], ExE[:, :], rsE[:, :], None, ALU.mult)

    tpv = psm.tile([128, BC], F32, tag="sm", name="tpv")
    nc.tensor.transpose(tpv[:, :], eout[:, :], identF[0:BC, 0:BC])
    vT = wtile([128, BC], F32R, "vT")
    nc.vector.tensor_copy(vT[:, :], tpv[:, :])
    _tap(nc, io, "vT", vT[:, :])

    # ---------------- manipulator ----------------
    cxs = {}
    for i, v in enumerate(("int", "h0", "hL")):
        cx = psm.tile([64, BC], F32, tag="sm", name=f"cx{v}")
        nc.tensor.matmul(cx[:, :], wsumT[:, i * 64:(i + 1) * 64], vT[:, :],
                         start=True, stop=True)
        cxs[v] = wtile([64, BC], F32R, f"cxs_{v}")
        nc.scalar.activation(cxs[v][:, :], cx[:, :], AF.Relu, bias=mcb_col[:, :])
    mp = plin.tile([BC, 256], F32, tag="lp", name="mp")
    for i, v in enumerate(("int", "h0", "hL")):
        nc.tensor.matmul(mp[:, :], cxs[v][:, :], mlwS[:, i * 256:(i + 1) * 256],
                         start=(i == 0), stop=(i == 2))
    m_sb = wtile([BC, 256], F32, "m_sb")
    nc.vector.tensor_tensor(m_sb[:, :], mp[:, :], mlbB[:, :], ALU.add)
    _tap(nc, io, "m", m_sb[:, :])

    # tokens = floor(|m|*100) mod 14; pair idx = 14*even + odd
    # floor via the 2^23 magic-number trick; mod 14 via 2 conditional subtracts
    tt = wtile([BC, 256], F32, "tt")
    nc.scalar.activation(tt[:, :], m_sb[:, :], AF.Abs, scale=100.0)
    fu = wtile([BC, 256], F32, "fu")
    nc.vector.tensor_scalar(fu[:, :], tt[:, :], 8388607.5, None, ALU.add)
    fr = wtile([BC, 256], F32, "fr")
    nc.vector.tensor_scalar(fr[:, :], fu[:, :], 8388608.0, None, ALU.subtract)
    ti = wtile([BC, 256], F32, "ti")
    nc.vector.tensor_scalar(ti[:, :], fr[:, :], float(V), None, ALU.is_ge)
    t1 = wtile([BC, 256], F32, "t1")
    nc.vector.scalar_tensor_tensor(t1[:, :], ti[:, :], -float(V), fr[:, :],
                                   ALU.mult, ALU.add)
    t2 = wtile([BC, 256], F32, "t2")
    nc.vector.tensor_scalar(t2[:, :], t1[:, :], float(V), None, ALU.is_ge)
    tok = wtile([BC, 256], F32, "tok")
    nc.vector.scalar_tensor_tensor(tok[:, :], t2[:, :], -float(V), t1[:, :],
                                   ALU.mult, ALU.add)
    _tap(nc, io, "tok", tok[:, :])
    idxF = wtile([BC, H], BF16, "idxF")
    nc.vector.scalar_tensor_tensor(idxF[:, :], tok[:, 0:256:2], float(V),
                                   tok[:, 1:256:2], ALU.mult, ALU.add)
    idxrowF = wtile([1, BC * H], BF16, "idxrowF")
    nc.gpsimd.dma_start(idxrowF[:, :], idxF[:, :])

    # ---------------- friend branch ----------------
    ohF0, ohF1 = build_oh(idxrowF, "F")
    actsF = conv_apply(ohF0, ohF1, cwF0, cwF1, "F")
    lpF = big_linear(actsF, flwP, "F")
    fsb = wtile([BC, 128], F32, "fsb")
    nc.vector.tensor_tensor(fsb[:, :], lpF[:, :], flbeB[:, :], ALU.add)

    tpf = psm.tile([128, BC], F32, tag="sm", name="tpf")
    nc.tensor.transpose(tpf[:, :], fsb[:, :], identF[0:BC, 0:BC])
    fT = wtile([128, BC], F32, "fT")
    nc.vector.tensor_copy(fT[:, :], tpf[:, :])
    f2 = psm.tile([BC, 14], F32, tag="sm", name="f2")
    nc.tensor.matmul(f2[:, :], fT[:, :], w2sb[:, :], start=True, stop=True)
    logits = wtile([BC, 14], F32, "logits")
    nc.vector.tensor_tensor(logits[:, :], f2[:, :], f2bB[:, :], ALU.add)
    nmx = wtile([BC, 1], F32, "nmx")
    nc.vector.reduce_max(nmx[:, :], logits[:, :], AX.X, negate=True)
    ex = wtile([BC, 14], F32, "ex")
    nc.scalar.activation(ex[:, :], logits[:, :], AF.Exp, bias=nmx[:, :])
    sm = wtile([BC, 1], F32, "sm")
    nc.vector.reduce_sum(sm[:, :], ex[:, :], AX.X)
    rs = wtile([BC, 1], F32, "rs")
    nc.vector.reciprocal(rs[:, :], sm[:, :])
    outt = wtile([BC, 14], F32, "outt")
    nc.vector.tensor_scalar(outt[:, :], ex[:, :], rs[:, :], None, ALU.mult)
    nc.sync.dma_start(io["out"], outt[:, :])


_CACHE = {}


def _get_nc():
    if "nc" not in _CACHE:
        nc = bacc.Bacc("TRN2", target_bir_lowering=False, debug=False,
                       num_devices=NCORES)
        with tile.TileContext(nc) as tc:
            with ExitStack() as ctx:
                build_kernel(nc, tc, ctx)
        nc.compile()
        _CACHE["nc"] = nc
    return _CACHE["nc"]


def prep_inputs(inputs):
    """Host-side composition + shard. Returns list of 8 in_maps."""
    f32 = np.float32
    bf16 = ml_dtypes.bfloat16

    def cw_tables(emb, cw_full):
        emb = np.asarray(emb, f32)
        cw = np.ascontiguousarray(np.asarray(cw_full, f32)[:, :, :, 1])  # [O,I,3]
        t0, t1 = np.meshgrid(np.arange(V), np.arange(V), indexing="ij")
        table = np.maximum(emb[t0.ravel()], emb[t1.ravel()])            # [196,512]
        cwc = np.concatenate([table @ cw[:, :, kh].T for kh in range(3)],
                             axis=1).astype(bf16)                        # [196,768]
        return np.ascontiguousarray(cwc[:P0]), np.ascontiguousarray(cwc[P0:])

    cwE0, cwE1 = cw_tables(inputs["enemy_emb"], inputs["enemy_conv_w"])
    cwF0, cwF1 = cw_tables(inputs["friend_emb"], inputs["friend_conv_w"])

    elw = np.asarray(inputs["enemy_lin_w"], f32)
    flw = np.asarray(inputs["friend_lin1_w"], f32)
    elbe = (np.asarray(inputs["enemy_lin_b"], f32)
            + np.einsum("o,ohj->j", np.asarray(inputs["enemy_conv_b"], f32),
                        elw.reshape(256, 128, 128), optimize=True)).astype(f32)
    flbe = (np.asarray(inputs["friend_lin1_b"], f32)
            + np.einsum("o,ohj->j", np.asarray(inputs["friend_conv_b"], f32),
                        flw.reshape(256, 128, 128), optimize=True)).astype(f32)

    mcw = np.asarray(inputs["manip_conv_w"], f32)[:, :, :, 1]  # [64,128,3]
    s_int = mcw.sum(2)
    s12 = mcw[:, :, 1] + mcw[:, :, 2]
    s01 = mcw[:, :, 0] + mcw[:, :, 1]
    wsumT = np.concatenate([s_int.T, s12.T, s01.T], axis=1).astype(f32)  # [128,192]

    mlw3 = np.asarray(inputs["manip_lin_w"], f32).reshape(64, 128, 256)
    mlwS = np.concatenate([mlw3[:, 1:127].sum(1), mlw3[:, 0], mlw3[:, 127]],
                          axis=1).astype(f32)                            # [64,768]

    common = {
        "cwE0": cwE0, "cwE1": cwE1, "cwF0": cwF0, "cwF1": cwF1,
        "elw3": np.ascontiguousarray(elw.reshape(256, 128 * 128)).astype(bf16),
        "flw3": np.ascontiguousarray(flw.reshape(256, 128 * 128)).astype(bf16),
        "mlwS": np.ascontiguousarray(mlwS),
        "wsumT": np.ascontiguousarray(wsumT),
        "mcb": np.ascontiguousarray(inputs["manip_conv_b"], f32),
        "elbe": elbe,
        "flbe": flbe,
        "mlb": np.ascontiguousarray(inputs["manip_lin_b"], f32),
        "f2w": np.ascontiguousarray(inputs["friend_lin2_w"], f32),
        "f2b": np.ascontiguousarray(inputs["friend_lin2_b"], f32),
    }
    x = np.ascontiguousarray(np.asarray(inputs["x"], np.int32))
    return [dict(common, x=np.ascontiguousarray(x[c * BC:(c + 1) * BC]))
            for c in range(NCORES)]


def kernel(**inputs):
    nc = _get_nc()
    in_maps = prep_inputs(inputs)
    res = run_bass_kernel_spmd(nc, in_maps, core_ids=list(range(NCORES)))
    return np.concatenate([r["out"] for r in res.results], axis=0)


# revision 18
# speedup vs baseline: 5.2891x; 1.0639x over previous
"""Trainium2 Bass kernel for nn_Network_67388036874689.

Data-parallel over batch: B=256 sharded as 32 samples on each of 8 cores;
all parameters replicated (host-precomposed).

Structure exploited (validated against the reference on host):
  - fog_of_war's greedy scan returns arange(B) -> the permutation is identity.
  - conv2d(3x3, pad=1) on [C, H, 1] spatial input only sees kernel column 1
    -> 1D conv over H with 3 taps.
  - Embedding (V=14) + pair-maxpool + conv compose into per-tap tables
    CW[kh] = pairmax_table @ conv_w[:, :, kh].T  (196 x 256), host-built.
    Device conv = one-hot(pair idx) matmuls against CW with +-1 shifts.
  - Conv bias folds into the following linear's bias (host).
  - The manipulator conv input is constant over h -> the 8192x256 manip
    linear collapses to 3 reduced 64x256 matrices (host-summed over h).

Precision: tables/linears in bf16 (host sim: 2/65536 token flips,
rel err ~1e-4); manipulator path f32/f32r; all psum accumulation f32.
"""

import numpy as np
import ml_dtypes
from contextlib import ExitStack

import concourse.bass as bass
import concourse.bacc as bacc
import concourse.mybir as mybir
import concourse.tile as tile
from concourse.masks import make_identity
from concourse.bass_utils import run_bass_kernel_spmd

F32 = mybir.dt.float32
F32R = mybir.dt.float32r
BF16 = mybir.dt.bfloat16
I32 = mybir.dt.int32
AF = mybir.ActivationFunctionType
ALU = mybir.AluOpType
AX = mybir.AxisListType

NCORES = 8
B = 256
BC = B // NCORES        # 32 samples per core
L = 256
V = 14
EMB = 512
H = L // 2              # 128 pooled positions
NPAIR = V * V           # 196
P0 = 112                # pair-table partition split: 112 + 84
P1 = NPAIR - P0
SW = H + 2              # 130: per-sample padded width in the one-hot tiles
OHW = BC * SW           # 4160
DEBUG_TAPS = False


def _dram_inputs(nc):
    t = {}

    def inp(name, shape, dt):
        t[name] = nc.dram_tensor(name, list(shape), dt, kind="ExternalInput").ap()

    inp("idxrowE", (1, BC * H), BF16)   # host: 14*x[:, 0::2] + x[:, 1::2], flat
    inp("cwE0", (P0, 768), BF16)     # enemy CW tables, col = kh*256 + o
    inp("cwE1", (P1, 768), BF16)
    inp("cwF0", (P0, 768), BF16)
    inp("cwF1", (P1, 768), BF16)
    inp("elw3", (256, 128 * 128), BF16)   # [o, (h, j)]
    inp("flw3", (256, 128 * 128), BF16)
    inp("mlwS", (64, 768), F32R)     # col = v*256 + j, v in (int, h0, hL)
    inp("wsumT", (128, 192), F32R)   # col = v*64 + o
    inp("mcb", (64,), F32)
    inp("elbe", (128,), F32)         # enemy lin bias + folded conv bias
    inp("flbe", (128,), F32)
    inp("mlb", (256,), F32)
    inp("f2w", (128, 14), F32)
    inp("f2b", (14,), F32)
    t["out"] = nc.dram_tensor("out", [BC, 14], F32, kind="ExternalOutput").ap()
    return t


def _tap(nc, io, name, ap):
    if not DEBUG_TAPS:
        return
    t = nc.dram_tensor("tap_" + name, list(ap.shape), ap.dtype,
                       kind="ExternalOutput").ap()
    io["tap_" + name] = t
    nc.gpsimd.dma_start(t, ap)


def build_kernel(nc, tc, ctx):
    io = _dram_inputs(nc)
    consts = ctx.enter_context(tc.tile_pool(name="consts", bufs=1))
    work = ctx.enter_context(tc.tile_pool(name="work", bufs=1))
    wpool = ctx.enter_context(tc.tile_pool(name="wstream", bufs=6))
    ohpool = ctx.enter_context(tc.tile_pool(name="ohpool", bufs=1))
    ppp = ctx.enter_context(tc.tile_pool(name="ppp", bufs=2, space="PSUM"))
    pconv = ctx.enter_context(tc.tile_pool(name="pconv", bufs=4, space="PSUM"))
    plin = ctx.enter_context(tc.tile_pool(name="plin", bufs=1, space="PSUM"))
    psm = ctx.enter_context(tc.tile_pool(name="psm", bufs=1, space="PSUM"))

    def ctile(shape, dt, tag):
        return consts.tile(shape, dt, tag=tag, name=tag)

    def wtile(shape, dt, tag):
        return work.tile(shape, dt, tag=tag, name=tag)

    # ---------------- constants & small weights ----------------
    identF = ctile([128, 128], F32, "identF")
    make_identity(nc, identF)
    iota_i = ctile([128, 1], I32, "iota_i")
    nc.gpsimd.iota(iota_i[:, :], pattern=[[0, 1]], base=0, channel_multiplier=1)
    iota_col = ctile([128, 1], F32, "iota_col")
    nc.vector.tensor_copy(iota_col[:, :], iota_i[:, :])
    ones_row = ctile([1, 128], BF16, "ones_row")
    nc.vector.memset(ones_row[:, :], 1.0)

    def bias_col(dram_vec, n, tag):
        col = ctile([n, 1], F32, tag)
        nc.gpsimd.dma_start(col[:, :], dram_vec)
        return col

    def bias_bcast(dram_vec, rows, width, tag):
        out = ctile([rows, width], F32, tag)
        nc.gpsimd.dma_start(out[:, :], dram_vec[None, :].partition_broadcast(rows))
        return out

    # enemy pair-index row first on the sync HWDGE ring (host-precomputed):
    # the whole front of the kernel needs it
    idxrowE = wtile([1, BC * H], BF16, "idxrowE")
    nc.sync.dma_start(idxrowE[:, :], io["idxrowE"])

    elbeB = bias_bcast(io["elbe"], BC, 128, "elbeB")
    flbeB = bias_bcast(io["flbe"], BC, 128, "flbeB")
    mlbB = bias_bcast(io["mlb"], BC, 256, "mlbB")
    f2bB = bias_bcast(io["f2b"], BC, 14, "f2bB")
    mcb_col = bias_col(io["mcb"], 64, "mcb")

    def load(name, shape, dt):
        t = ctile(shape, dt, name)
        nc.sync.dma_start(t[:, :], io[name])
        return t

    cwE0 = load("cwE0", [P0, 768], BF16)
    cwE1 = load("cwE1", [P1, 768], BF16)
    cwF0 = load("cwF0", [P0, 768], BF16)
    cwF1 = load("cwF1", [P1, 768], BF16)
    mlwS = load("mlwS", [64, 768], F32R)
    wsumT = load("wsumT", [128, 192], F32R)
    w2sb = load("f2w", [128, 14], F32)

    # ---------------- streamed big weights ----------------
    # elw3/flw3: [o(256), (h,j)]; piece = [o-half(128), 64 h x 128 j] = 2 MB.
    # On the scalar-engine HWDGE ring so they don't queue behind (or ahead
    # of) the small sync-ring loads.
    def stream_weights(dram):
        pieces = []
        for half in range(2):
            for hb in range(2):
                p = wpool.tile([128, 64 * 128], BF16, tag="wp", name="wp")
                nc.scalar.dma_start(
                    p[:, :], dram[half * 128:(half + 1) * 128,
                                  hb * 8192:(hb + 1) * 8192])
                pieces.append(p)
        return pieces

    elwP = stream_weights(io["elw3"])
    flwP = stream_weights(io["flw3"])

    # ---------------- stage helpers ----------------
    def build_oh(idxrow, tag):
        """One-hot over the pair-idx row [1, 4096] (col s*128+h), padded
        layout: col s*130 + 1 + h holds [idx[s,h] == t]; cols s*130 and
        s*130+129 are zero (conv boundary)."""
        oh0 = ohpool.tile([P0, OHW], BF16, tag="oh0", name=f"oh0{tag}")
        oh1 = ohpool.tile([P1, OHW], BF16, tag="oh1", name=f"oh1{tag}")
        for oh in (oh0, oh1):
            nc.vector.memset(oh[:, 0:OHW:SW], 0.0)
            nc.vector.memset(oh[:, SW - 1:OHW:SW], 0.0)
        for blk in range(8):
            pp = ppp.tile([P0, 512], F32, tag="pp", name="pp")
            nc.tensor.matmul(pp[:, :], ones_row[:, 0:P0],
                             idxrow[:, blk * 512:(blk + 1) * 512],
                             start=True, stop=True)
            src = pp[:, :].rearrange("p (s w) -> p s w", w=128)
            dst0 = oh0[:, blk * 4 * SW:(blk + 1) * 4 * SW] \
                .rearrange("p (s w) -> p s w", w=SW)[:, :, 1:129]
            nc.vector.tensor_scalar(dst0, src, iota_col[0:P0, :], None,
                                    ALU.is_equal)
            dst1 = oh1[:, blk * 4 * SW:(blk + 1) * 4 * SW] \
                .rearrange("p (s w) -> p s w", w=SW)[:, :, 1:129]
            nc.vector.tensor_scalar(dst1, src[0:P1], float(P0),
                                    iota_col[0:P1, :], ALU.subtract,
                                    ALU.is_equal)
        return oh0, oh1

    def conv_apply(oh0, oh1, cw0, cw1, tag):
        """y[o, (s,h)] = sum_kh CW_kh[idx[h+kh-1], o]; acts as 2 halves
        [128 o', 32*128 (s,h)] bf16."""
        acts = [wtile([128, BC * H], BF16, f"acts{tag}{oc}") for oc in range(2)]
        for oc in range(2):
            for blk in range(8):
                cp = pconv.tile([128, 512], F32, tag="cp", name="cp")
                n = 0
                for cw, oh, npart in ((cw0, oh0, P0), (cw1, oh1, P1)):
                    for kh in range(3):
                        lhsT = cw[:, kh * 256 + oc * 128:
                                  kh * 256 + (oc + 1) * 128]
                        rhs = oh[:, blk * 4 * SW:(blk + 1) * 4 * SW] \
                            .rearrange("p (s w) -> p s w", w=SW)[:, :, kh:kh + 128]
                        nc.tensor.matmul(cp[:, :], lhsT, rhs,
                                         start=(n == 0), stop=(n == 5))
                        n += 1
                dst = acts[oc][:, blk * 512:(blk + 1) * 512]
                if blk % 2 == 0:
                    nc.scalar.activation(dst, cp[:, :], AF.Copy)
                else:
                    nc.vector.tensor_copy(dst, cp[:, :])
        return acts

    def big_linear(acts, pieces, tag):
        """lp[s, j] = sum_{o,h} acts[o][:, s*128+h] * W[(o,h), j]"""
        lp = plin.tile([BC, 128], F32, tag="lp", name=f"lp{tag}")
        for half in range(2):
            for h in range(128):
                piece = pieces[half * 2 + h // 64]
                lhsT = acts[half][:, h:h + (BC - 1) * 128 + 1:128]
                rhs = piece[:, (h % 64) * 128:(h % 64 + 1) * 128]
                nc.tensor.matmul(lp[:, :], lhsT, rhs,
                                 start=(half == 0 and h == 0),
                                 stop=(half == 1 and h == 127))
        return lp

    # ---------------- enemy branch ----------------
    ohE0, ohE1 = build_oh(idxrowE, "E")
    actsE = conv_apply(ohE0, ohE1, cwE0, cwE1, "E")
    _tap(nc, io, "actsE0", actsE[0][:, :])
    lpE = big_linear(actsE, elwP, "E")

    logitsE = wtile([BC, 128], F32, "logitsE")
    nc.vector.tensor_tensor(logitsE[:, :], lpE[:, :], elbeB[:, :], ALU.add)
    _tap(nc, io, "logitsE", logitsE[:, :])
    nmxE = wtile([BC, 1], F32, "nmxE")
    nc.vector.reduce_max(nmxE[:, :], logitsE[:, :], AX.X, negate=True)
    ExE = wtile([BC, 128], F32, "ExE")
    nc.scalar.activation(ExE[:, :], logitsE[:, :], AF.Exp, bias=nmxE[:, :])
    smE = wtile([BC, 1], F32, "smE")
    nc.vector.reduce_sum(smE[:, :], ExE[:, :], AX.X)
    rsE = wtile([BC, 1], F32, "rsE")
    nc.vector.reciprocal(rsE[:, :], smE[:, :])
    eout = wtile([BC, 128], F32, "eout")
    nc.vector.tensor_scalar(eout[:, :], ExE[:, :], rsE[:, :], None, ALU.mult)

    tpv = psm.tile([128, BC], F32, tag="sm", name="tpv")
    nc.tensor.transpose(tpv[:, :], eout[:, :], identF[0:BC, 0:BC])
    vT = wtile([128, BC], F32R, "vT")
    nc.vector.tensor_copy(vT[:, :], tpv[:, :])
    _tap(nc, io, "vT", vT[:, :])

    # ---------------- manipulator ----------------
    cxs = {}
    for i, v in enumerate(("int", "h0", "hL")):
        cx = psm.tile([64, BC], F32, tag="sm", name=f"cx{v}")
        nc.tensor.matmul(cx[:, :], wsumT[:, i * 64:(i + 1) * 64], vT[:, :],
                         start=True, stop=True)
        cxs[v] = wtile([64, BC], F32R, f"cxs_{v}")
        nc.scalar.activation(cxs[v][:, :], cx[:, :], AF.Relu, bias=mcb_col[:, :])
    mp = plin.tile([BC, 256], F32, tag="lp", name="mp")
    for i, v in enumerate(("int", "h0", "hL")):
        nc.tensor.matmul(mp[:, :], cxs[v][:, :], mlwS[:, i * 256:(i + 1) * 256],
                         start=(i == 0), stop=(i == 2))
    m_sb = wtile([BC, 256], F32, "m_sb")
    nc.vector.tensor_tensor(m_sb[:, :], mp[:, :], mlbB[:, :], ALU.add)
    _tap(nc, io, "m", m_sb[:, :])

    # tokens = floor(|m|*100) mod 14; pair idx = 14*even + odd
    # floor via the 2^23 magic-number trick; mod 14 via 2 conditional subtracts
    tt = wtile([BC, 256], F32, "tt")
    nc.scalar.activation(tt[:, :], m_sb[:, :], AF.Abs, scale=100.0)
    fu = wtile([BC, 256], F32, "fu")
    nc.vector.tensor_scalar(fu[:, :], tt[:, :], 8388607.5, None, ALU.add)
    fr = wtile([BC, 256], F32, "fr")
    nc.vector.tensor_scalar(fr[:, :], fu[:, :], 8388608.0, None, ALU.subtract)
    ti = wtile([BC, 256], F32, "ti")
    nc.vector.tensor_scalar(ti[:, :], fr[:, :], float(V), None, ALU.is_ge)
    t1 = wtile([BC, 256], F32, "t1")
    nc.vector.scalar_tensor_tensor(t1[:, :], ti[:, :], -float(V), fr[:, :],
                                   ALU.mult, ALU.add)
    t2 = wtile([BC, 256], F32, "t2")
    nc.vector.tensor_scalar(t2[:, :], t1[:, :], float(V), None, ALU.is_ge)
    tok = wtile([BC, 256], F32, "tok")
    nc.vector.scalar_tensor_tensor(tok[:, :], t2[:, :], -float(V), t1[:, :],
                                   ALU.mult, ALU.add)
    _tap(nc, io, "tok", tok[:, :])
    idxF = wtile([BC, H], BF16, "idxF")
    nc.vector.scalar_tensor_tensor(idxF[:, :], tok[:, 0:256:2], float(V),
                                   tok[:, 1:256:2], ALU.mult, ALU.add)
    idxrowF = wtile([1, BC * H], BF16, "idxrowF")
    nc.sync.dma_start(idxrowF[:, :], idxF[:, :])

    # ---------------- friend branch ----------------
    ohF0, ohF1 = build_oh(idxrowF, "F")
    actsF = conv_apply(ohF0, ohF1, cwF0, cwF1, "F")
    lpF = big_linear(actsF, flwP, "F")
    fsb = wtile([BC, 128], F32, "fsb")
    nc.vector.tensor_tensor(fsb[:, :], lpF[:, :], flbeB[:, :], ALU.add)

    tpf = psm.tile([128, BC], F32, tag="sm", name="tpf")
    nc.tensor.transpose(tpf[:, :], fsb[:, :], identF[0:BC, 0:BC])
    fT = wtile([128, BC], F32, "fT")
    nc.vector.tensor_copy(fT[:, :], tpf[:, :])
    f2 = psm.tile([BC, 14], F32, tag="sm", name="f2")
    nc.tensor.matmul(f2[:, :], fT[:, :], w2sb[:, :], start=True, stop=True)
    logits = wtile([BC, 14], F32, "logits")
    nc.vector.tensor_tensor(logits[:, :], f2[:, :], f2bB[:, :], ALU.add)
    nmx = wtile([BC, 1], F32, "nmx")
    nc.vector.reduce_max(nmx[:, :], logits[:, :], AX.X, negate=True)
    ex = wtile([BC, 14], F32, "ex")
    nc.scalar.activation(ex[:, :], logits[:, :], AF.Exp, bias=nmx[:, :])
    sm = wtile([BC, 1], F32, "sm")
    nc.vector.reduce_sum(sm[:, :], ex[:, :], AX.X)
    rs = wtile([BC, 1], F32, "rs")
    nc.vector.reciprocal(rs[:, :], sm[:, :])
    outt = wtile([BC, 14], F32, "outt")
    nc.vector.tensor_scalar(outt[:, :], ex[:, :], rs[:, :], None, ALU.mult)
    nc.sync.dma_start(io["out"], outt[:, :])


_CACHE = {}


def _get_nc():
    if "nc" not in _CACHE:
        nc = bacc.Bacc("TRN2", target_bir_lowering=False, debug=False,
                       num_devices=NCORES)
        with tile.TileContext(nc) as tc:
            with ExitStack() as ctx:
                build_kernel(nc, tc, ctx)
        nc.compile()
        _CACHE["nc"] = nc
    return _CACHE["nc"]


def prep_inputs(inputs):
    """Host-side composition + shard. Returns list of 8 in_maps."""
    f32 = np.float32
    bf16 = ml_dtypes.bfloat16

    def cw_tables(emb, cw_full):
        emb = np.asarray(emb, f32)
        cw = np.ascontiguousarray(np.asarray(cw_full, f32)[:, :, :, 1])  # [O,I,3]
        t0, t1 = np.meshgrid(np.arange(V), np.arange(V), indexing="ij")
        table = np.maximum(emb[t0.ravel()], emb[t1.ravel()])            # [196,512]
        cwc = np.concatenate([table @ cw[:, :, kh].T for kh in range(3)],
                             axis=1).astype(bf16)                        # [196,768]
        return np.ascontiguousarray(cwc[:P0]), np.ascontiguousarray(cwc[P0:])

    cwE0, cwE1 = cw_tables(inputs["enemy_emb"], inputs["enemy_conv_w"])
    cwF0, cwF1 = cw_tables(inputs["friend_emb"], inputs["friend_conv_w"])

    elw = np.asarray(inputs["enemy_lin_w"], f32)
    flw = np.asarray(inputs["friend_lin1_w"], f32)
    elbe = (np.asarray(inputs["enemy_lin_b"], f32)
            + np.einsum("o,ohj->j", np.asarray(inputs["enemy_conv_b"], f32),
                        elw.reshape(256, 128, 128), optimize=True)).astype(f32)
    flbe = (np.asarray(inputs["friend_lin1_b"], f32)
            + np.einsum("o,ohj->j", np.asarray(inputs["friend_conv_b"], f32),
                        flw.reshape(256, 128, 128), optimize=True)).astype(f32)

    mcw = np.asarray(inputs["manip_conv_w"], f32)[:, :, :, 1]  # [64,128,3]
    s_int = mcw.sum(2)
    s12 = mcw[:, :, 1] + mcw[:, :, 2]
    s01 = mcw[:, :, 0] + mcw[:, :, 1]
    wsumT = np.concatenate([s_int.T, s12.T, s01.T], axis=1).astype(f32)  # [128,192]

    mlw3 = np.asarray(inputs["manip_lin_w"], f32).reshape(64, 128, 256)
    mlwS = np.concatenate([mlw3[:, 1:127].sum(1), mlw3[:, 0], mlw3[:, 127]],
                          axis=1).astype(f32)                            # [64,768]

    common = {
        "cwE0": cwE0, "cwE1": cwE1, "cwF0": cwF0, "cwF1": cwF1,
        "elw3": np.ascontiguousarray(elw.reshape(256, 128 * 128)).astype(bf16),
        "flw3": np.ascontiguousarray(flw.reshape(256, 128 * 128)).astype(bf16),
        "mlwS": np.ascontiguousarray(mlwS),
        "wsumT": np.ascontiguousarray(wsumT),
        "mcb": np.ascontiguousarray(inputs["manip_conv_b"], f32),
        "elbe": elbe,
        "flbe": flbe,
        "mlb": np.ascontiguousarray(inputs["manip_lin_b"], f32),
        "f2w": np.ascontiguousarray(inputs["friend_lin2_w"], f32),
        "f2b": np.ascontiguousarray(inputs["friend_lin2_b"], f32),
    }
    x = np.asarray(inputs["x"], np.int64)
    idxrow = (V * x[:, 0::2] + x[:, 1::2]).astype(bf16)   # [B, 128], ints < 196
    return [dict(common,
                 idxrowE=np.ascontiguousarray(
                     idxrow[c * BC:(c + 1) * BC].reshape(1, BC * H)))
            for c in range(NCORES)]


def kernel(**inputs):
    nc = _get_nc()
    in_maps = prep_inputs(inputs)
    res = run_bass_kernel_spmd(nc, in_maps, core_ids=list(range(NCORES)))
    return np.concatenate([r["out"] for r in res.results], axis=0)


# revision 19
# speedup vs baseline: 6.0459x; 1.1431x over previous
"""Trainium2 Bass kernel for nn_Network_67388036874689.

Data-parallel over batch: B=256 sharded as 32 samples on each of 8 cores;
all parameters replicated (host-precomposed).

Structure exploited (validated against the reference on host):
  - fog_of_war's greedy scan returns arange(B) -> the permutation is identity.
  - conv2d(3x3, pad=1) on [C, H, 1] spatial input only sees kernel column 1
    -> 1D conv over H with 3 taps.
  - Embedding (V=14) + pair-maxpool + conv compose into per-tap tables
    CW[kh] = pairmax_table @ conv_w[:, :, kh].T  (196 x 256), host-built.
    Device conv = one-hot(pair idx) matmuls against CW with +-1 shifts.
  - Conv bias folds into the following linear's bias (host).
  - The manipulator conv input is constant over h -> the 8192x256 manip
    linear collapses to 3 reduced 64x256 matrices (host-summed over h).

Precision: tables/linears in bf16 (host sim: 2/65536 token flips,
rel err ~1e-4); manipulator path f32/f32r; all psum accumulation f32.
"""

import numpy as np
import ml_dtypes
from contextlib import ExitStack

import concourse.bass as bass
import concourse.bacc as bacc
import concourse.mybir as mybir
import concourse.tile as tile
from concourse.masks import make_identity
from concourse.bass_utils import run_bass_kernel_spmd

F32 = mybir.dt.float32
F32R = mybir.dt.float32r
BF16 = mybir.dt.bfloat16
I32 = mybir.dt.int32
AF = mybir.ActivationFunctionType
ALU = mybir.AluOpType
AX = mybir.AxisListType

NCORES = 8
B = 256
BC = B // NCORES        # 32 samples per core
L = 256
V = 14
EMB = 512
H = L // 2              # 128 pooled positions
NPAIR = V * V           # 196
P0 = 112                # pair-table partition split: 112 + 84
P1 = NPAIR - P0
SW = H + 2              # 130: per-sample padded width in the one-hot tiles
OHW = BC * SW           # 4160
DEBUG_TAPS = False


def _dram_inputs(nc):
    t = {}

    def inp(name, shape, dt):
        t[name] = nc.dram_tensor(name, list(shape), dt, kind="ExternalInput").ap()

    inp("idxrowE", (1, BC * H), BF16)   # host: 14*x[:, 0::2] + x[:, 1::2], flat
    inp("cwE0", (P0, 768), BF16)     # enemy CW tables, col = kh*256 + o
    inp("cwE1", (P1, 768), BF16)
    inp("cwF0", (P0, 768), BF16)
    inp("cwF1", (P1, 768), BF16)
    inp("elw3", (256, 128 * 128), BF16)   # [o, (h, j)]
    inp("flw3", (256, 128 * 128), BF16)
    inp("mlwS", (64, 768), F32R)     # col = v*256 + j, v in (int, h0, hL)
    inp("wsumT", (128, 192), F32R)   # col = v*64 + o
    inp("mcb", (64,), F32)
    inp("elbe", (128,), F32)         # enemy lin bias + folded conv bias
    inp("flbe", (128,), F32)
    inp("mlb", (256,), F32)
    inp("f2w", (128, 14), F32)
    inp("f2b", (14,), F32)
    t["out"] = nc.dram_tensor("out", [BC, 14], F32, kind="ExternalOutput").ap()
    return t


def _tap(nc, io, name, ap):
    if not DEBUG_TAPS:
        return
    t = nc.dram_tensor("tap_" + name, list(ap.shape), ap.dtype,
                       kind="ExternalOutput").ap()
    io["tap_" + name] = t
    nc.gpsimd.dma_start(t, ap)


def build_kernel(nc, tc, ctx):
    io = _dram_inputs(nc)
    consts = ctx.enter_context(tc.tile_pool(name="consts", bufs=1))
    work = ctx.enter_context(tc.tile_pool(name="work", bufs=1))
    wpool = ctx.enter_context(tc.tile_pool(name="wstream", bufs=6))
    ohpool = ctx.enter_context(tc.tile_pool(name="ohpool", bufs=1))
    ppp = ctx.enter_context(tc.tile_pool(name="ppp", bufs=2, space="PSUM"))
    pconv = ctx.enter_context(tc.tile_pool(name="pconv", bufs=4, space="PSUM"))
    plin = ctx.enter_context(tc.tile_pool(name="plin", bufs=1, space="PSUM"))
    psm = ctx.enter_context(tc.tile_pool(name="psm", bufs=1, space="PSUM"))

    def ctile(shape, dt, tag):
        return consts.tile(shape, dt, tag=tag, name=tag)

    def wtile(shape, dt, tag):
        return work.tile(shape, dt, tag=tag, name=tag)

    # ---------------- constants & small weights ----------------
    identF = ctile([128, 128], F32, "identF")
    make_identity(nc, identF)
    iota_i = ctile([128, 1], I32, "iota_i")
    nc.gpsimd.iota(iota_i[:, :], pattern=[[0, 1]], base=0, channel_multiplier=1)
    iota_col = ctile([128, 1], F32, "iota_col")
    nc.vector.tensor_copy(iota_col[:, :], iota_i[:, :])
    ones_row = ctile([1, 128], BF16, "ones_row")
    nc.vector.memset(ones_row[:, :], 1.0)

    def bias_col(dram_vec, n, tag):
        col = ctile([n, 1], F32, tag)
        nc.gpsimd.dma_start(col[:, :], dram_vec)
        return col

    def bias_bcast(dram_vec, rows, width, tag):
        out = ctile([rows, width], F32, tag)
        nc.gpsimd.dma_start(out[:, :], dram_vec[None, :].partition_broadcast(rows))
        return out

    # enemy pair-index row first on the sync HWDGE ring (host-precomputed):
    # the whole front of the kernel needs it
    idxrowE = wtile([1, BC * H], BF16, "idxrowE")
    nc.sync.dma_start(idxrowE[:, :], io["idxrowE"])

    elbeB = bias_bcast(io["elbe"], BC, 128, "elbeB")
    flbeB = bias_bcast(io["flbe"], BC, 128, "flbeB")
    mlbB = bias_bcast(io["mlb"], BC, 256, "mlbB")
    f2bB = bias_bcast(io["f2b"], BC, 14, "f2bB")
    mcb_col = bias_col(io["mcb"], 64, "mcb")

    def load(name, shape, dt):
        t = ctile(shape, dt, name)
        nc.sync.dma_start(t[:, :], io[name])
        return t

    # All HBM loads go on the single sync HWDGE ring in exact consumption
    # order: ring FIFO means the small early loads fully drain before the
    # big weight streams start. (Splitting across rings lets the SDMA
    # engines' packet-granular round-robin starve the small-packet queue:
    # 1.5 KB vs 16 KB packets -> the conv tables took 13+ us to land.)
    # elw3/flw3: [o(256), (h,j)]; piece = [o-half(128), 64 h x 128 j] = 2 MB
    def stream_weights(dram):
        pieces = []
        for half in range(2):
            for hb in range(2):
                p = wpool.tile([128, 64 * 128], BF16, tag="wp", name="wp")
                nc.sync.dma_start(
                    p[:, :], dram[half * 128:(half + 1) * 128,
                                  hb * 8192:(hb + 1) * 8192])
                pieces.append(p)
        return pieces

    cwE0 = load("cwE0", [P0, 768], BF16)
    cwE1 = load("cwE1", [P1, 768], BF16)
    elwP = stream_weights(io["elw3"])
    wsumT = load("wsumT", [128, 192], F32R)
    mlwS = load("mlwS", [64, 768], F32R)
    cwF0 = load("cwF0", [P0, 768], BF16)
    cwF1 = load("cwF1", [P1, 768], BF16)
    flwP = stream_weights(io["flw3"])
    w2sb = load("f2w", [128, 14], F32)

    # ---------------- stage helpers ----------------
    def build_oh(idxrow, tag):
        """One-hot over the pair-idx row [1, 4096] (col s*128+h), padded
        layout: col s*130 + 1 + h holds [idx[s,h] == t]; cols s*130 and
        s*130+129 are zero (conv boundary)."""
        oh0 = ohpool.tile([P0, OHW], BF16, tag="oh0", name=f"oh0{tag}")
        oh1 = ohpool.tile([P1, OHW], BF16, tag="oh1", name=f"oh1{tag}")
        for oh in (oh0, oh1):
            nc.vector.memset(oh[:, 0:OHW:SW], 0.0)
            nc.vector.memset(oh[:, SW - 1:OHW:SW], 0.0)
        for blk in range(8):
            pp = ppp.tile([P0, 512], F32, tag="pp", name="pp")
            nc.tensor.matmul(pp[:, :], ones_row[:, 0:P0],
                             idxrow[:, blk * 512:(blk + 1) * 512],
                             start=True, stop=True)
            src = pp[:, :].rearrange("p (s w) -> p s w", w=128)
            dst0 = oh0[:, blk * 4 * SW:(blk + 1) * 4 * SW] \
                .rearrange("p (s w) -> p s w", w=SW)[:, :, 1:129]
            nc.vector.tensor_scalar(dst0, src, iota_col[0:P0, :], None,
                                    ALU.is_equal)
            dst1 = oh1[:, blk * 4 * SW:(blk + 1) * 4 * SW] \
                .rearrange("p (s w) -> p s w", w=SW)[:, :, 1:129]
            nc.vector.tensor_scalar(dst1, src[0:P1], float(P0),
                                    iota_col[0:P1, :], ALU.subtract,
                                    ALU.is_equal)
        return oh0, oh1

    def conv_apply(oh0, oh1, cw0, cw1, tag):
        """y[o, (s,h)] = sum_kh CW_kh[idx[h+kh-1], o]; acts as 2 halves
        [128 o', 32*128 (s,h)] bf16."""
        acts = [wtile([128, BC * H], BF16, f"acts{tag}{oc}") for oc in range(2)]
        for oc in range(2):
            for blk in range(8):
                cp = pconv.tile([128, 512], F32, tag="cp", name="cp")
                n = 0
                for cw, oh, npart in ((cw0, oh0, P0), (cw1, oh1, P1)):
                    for kh in range(3):
                        lhsT = cw[:, kh * 256 + oc * 128:
                                  kh * 256 + (oc + 1) * 128]
                        rhs = oh[:, blk * 4 * SW:(blk + 1) * 4 * SW] \
                            .rearrange("p (s w) -> p s w", w=SW)[:, :, kh:kh + 128]
                        nc.tensor.matmul(cp[:, :], lhsT, rhs,
                                         start=(n == 0), stop=(n == 5))
                        n += 1
                dst = acts[oc][:, blk * 512:(blk + 1) * 512]
                if blk % 2 == 0:
                    nc.scalar.activation(dst, cp[:, :], AF.Copy)
                else:
                    nc.vector.tensor_copy(dst, cp[:, :])
        return acts

    def big_linear(acts, pieces, tag):
        """lp[s, j] = sum_{o,h} acts[o][:, s*128+h] * W[(o,h), j]"""
        lp = plin.tile([BC, 128], F32, tag="lp", name=f"lp{tag}")
        for half in range(2):
            for h in range(128):
                piece = pieces[half * 2 + h // 64]
                lhsT = acts[half][:, h:h + (BC - 1) * 128 + 1:128]
                rhs = piece[:, (h % 64) * 128:(h % 64 + 1) * 128]
                nc.tensor.matmul(lp[:, :], lhsT, rhs,
                                 start=(half == 0 and h == 0),
                                 stop=(half == 1 and h == 127))
        return lp

    # ---------------- enemy branch ----------------
    ohE0, ohE1 = build_oh(idxrowE, "E")
    actsE = conv_apply(ohE0, ohE1, cwE0, cwE1, "E")
    _tap(nc, io, "actsE0", actsE[0][:, :])
    lpE = big_linear(actsE, elwP, "E")

    logitsE = wtile([BC, 128], F32, "logitsE")
    nc.vector.tensor_tensor(logitsE[:, :], lpE[:, :], elbeB[:, :], ALU.add)
    _tap(nc, io, "logitsE", logitsE[:, :])
    nmxE = wtile([BC, 1], F32, "nmxE")
    nc.vector.reduce_max(nmxE[:, :], logitsE[:, :], AX.X, negate=True)
    ExE = wtile([BC, 128], F32, "ExE")
    nc.scalar.activation(ExE[:, :], logitsE[:, :], AF.Exp, bias=nmxE[:, :])
    smE = wtile([BC, 1], F32, "smE")
    nc.vector.reduce_sum(smE[:, :], ExE[:, :], AX.X)
    rsE = wtile([BC, 1], F32, "rsE")
    nc.vector.reciprocal(rsE[:, :], smE[:, :])
    eout = wtile([BC, 128], F32, "eout")
    nc.vector.tensor_scalar(eout[:, :], ExE[:, :], rsE[:, :], None, ALU.mult)

    tpv = psm.tile([128, BC], F32, tag="sm", name="tpv")
    nc.tensor.transpose(tpv[:, :], eout[:, :], identF[0:BC, 0:BC])
    vT = wtile([128, BC], F32R, "vT")
    nc.vector.tensor_copy(vT[:, :], tpv[:, :])
    _tap(nc, io, "vT", vT[:, :])

    # ---------------- manipulator ----------------
    cxs = {}
    for i, v in enumerate(("int", "h0", "hL")):
        cx = psm.tile([64, BC], F32, tag="sm", name=f"cx{v}")
        nc.tensor.matmul(cx[:, :], wsumT[:, i * 64:(i + 1) * 64], vT[:, :],
                         start=True, stop=True)
        cxs[v] = wtile([64, BC], F32R, f"cxs_{v}")
        nc.scalar.activation(cxs[v][:, :], cx[:, :], AF.Relu, bias=mcb_col[:, :])
    mp = plin.tile([BC, 256], F32, tag="lp", name="mp")
    for i, v in enumerate(("int", "h0", "hL")):
        nc.tensor.matmul(mp[:, :], cxs[v][:, :], mlwS[:, i * 256:(i + 1) * 256],
                         start=(i == 0), stop=(i == 2))
    m_sb = wtile([BC, 256], F32, "m_sb")
    nc.vector.tensor_tensor(m_sb[:, :], mp[:, :], mlbB[:, :], ALU.add)
    _tap(nc, io, "m", m_sb[:, :])

    # tokens = floor(|m|*100) mod 14; pair idx = 14*even + odd
    # floor via the 2^23 magic-number trick; mod 14 via 2 conditional subtracts
    tt = wtile([BC, 256], F32, "tt")
    nc.scalar.activation(tt[:, :], m_sb[:, :], AF.Abs, scale=100.0)
    fu = wtile([BC, 256], F32, "fu")
    nc.vector.tensor_scalar(fu[:, :], tt[:, :], 8388607.5, None, ALU.add)
    fr = wtile([BC, 256], F32, "fr")
    nc.vector.tensor_scalar(fr[:, :], fu[:, :], 8388608.0, None, ALU.subtract)
    ti = wtile([BC, 256], F32, "ti")
    nc.vector.tensor_scalar(ti[:, :], fr[:, :], float(V), None, ALU.is_ge)
    t1 = wtile([BC, 256], F32, "t1")
    nc.vector.scalar_tensor_tensor(t1[:, :], ti[:, :], -float(V), fr[:, :],
                                   ALU.mult, ALU.add)
    t2 = wtile([BC, 256], F32, "t2")
    nc.vector.tensor_scalar(t2[:, :], t1[:, :], float(V), None, ALU.is_ge)
    tok = wtile([BC, 256], F32, "tok")
    nc.vector.scalar_tensor_tensor(tok[:, :], t2[:, :], -float(V), t1[:, :],
                                   ALU.mult, ALU.add)
    _tap(nc, io, "tok", tok[:, :])
    idxF = wtile([BC, H], BF16, "idxF")
    nc.vector.scalar_tensor_tensor(idxF[:, :], tok[:, 0:256:2], float(V),
                                   tok[:, 1:256:2], ALU.mult, ALU.add)
    idxrowF = wtile([1, BC * H], BF16, "idxrowF")
    nc.sync.dma_start(idxrowF[:, :], idxF[:, :])

    # ---------------- friend branch ----------------
    ohF0, ohF1 = build_oh(idxrowF, "F")
    actsF = conv_apply(ohF0, ohF1, cwF0, cwF1, "F")
    lpF = big_linear(actsF, flwP, "F")
    fsb = wtile([BC, 128], F32, "fsb")
    nc.vector.tensor_tensor(fsb[:, :], lpF[:, :], flbeB[:, :], ALU.add)

    tpf = psm.tile([128, BC], F32, tag="sm", name="tpf")
    nc.tensor.transpose(tpf[:, :], fsb[:, :], identF[0:BC, 0:BC])
    fT = wtile([128, BC], F32, "fT")
    nc.vector.tensor_copy(fT[:, :], tpf[:, :])
    f2 = psm.tile([BC, 14], F32, tag="sm", name="f2")
    nc.tensor.matmul(f2[:, :], fT[:, :], w2sb[:, :], start=True, stop=True)
    logits = wtile([BC, 14], F32, "logits")
    nc.vector.tensor_tensor(logits[:, :], f2[:, :], f2bB[:, :], ALU.add)
    nmx = wtile([BC, 1], F32, "nmx")
    nc.vector.reduce_max(nmx[:, :], logits[:, :], AX.X, negate=True)
    ex = wtile([BC, 14], F32, "ex")
    nc.scalar.activation(ex[:, :], logits[:, :], AF.Exp, bias=nmx[:, :])
    sm = wtile([BC, 1], F32, "sm")
    nc.vector.reduce_sum(sm[:, :], ex[:, :], AX.X)
    rs = wtile([BC, 1], F32, "rs")
    nc.vector.reciprocal(rs[:, :], sm[:, :])
    outt = wtile([BC, 14], F32, "outt")
    nc.vector.tensor_scalar(outt[:, :], ex[:, :], rs[:, :], None, ALU.mult)
    nc.sync.dma_start(io["out"], outt[:, :])


_CACHE = {}


def _get_nc():
    if "nc" not in _CACHE:
        nc = bacc.Bacc("TRN2", target_bir_lowering=False, debug=False,
                       num_devices=NCORES)
        with tile.TileContext(nc) as tc:
            with ExitStack() as ctx:
                build_kernel(nc, tc, ctx)
        nc.compile()
        _CACHE["nc"] = nc
    return _CACHE["nc"]


def prep_inputs(inputs):
    """Host-side composition + shard. Returns list of 8 in_maps."""
    f32 = np.float32
    bf16 = ml_dtypes.bfloat16

    def cw_tables(emb, cw_full):
        emb = np.asarray(emb, f32)
        cw = np.ascontiguousarray(np.asarray(cw_full, f32)[:, :, :, 1])  # [O,I,3]
        t0, t1 = np.meshgrid(np.arange(V), np.arange(V), indexing="ij")
        table = np.maximum(emb[t0.ravel()], emb[t1.ravel()])            # [196,512]
        cwc = np.concatenate([table @ cw[:, :, kh].T for kh in range(3)],
                             axis=1).astype(bf16)                        # [196,768]
        return np.ascontiguousarray(cwc[:P0]), np.ascontiguousarray(cwc[P0:])

    cwE0, cwE1 = cw_tables(inputs["enemy_emb"], inputs["enemy_conv_w"])
    cwF0, cwF1 = cw_tables(inputs["friend_emb"], inputs["friend_conv_w"])

    elw = np.asarray(inputs["enemy_lin_w"], f32)
    flw = np.asarray(inputs["friend_lin1_w"], f32)
    elbe = (np.asarray(inputs["enemy_lin_b"], f32)
            + np.einsum("o,ohj->j", np.asarray(inputs["enemy_conv_b"], f32),
                        elw.reshape(256, 128, 128), optimize=True)).astype(f32)
    flbe = (np.asarray(inputs["friend_lin1_b"], f32)
            + np.einsum("o,ohj->j", np.asarray(inputs["friend_conv_b"], f32),
                        flw.reshape(256, 128, 128), optimize=True)).astype(f32)

    mcw = np.asarray(inputs["manip_conv_w"], f32)[:, :, :, 1]  # [64,128,3]
    s_int = mcw.sum(2)
    s12 = mcw[:, :, 1] + mcw[:, :, 2]
    s01 = mcw[:, :, 0] + mcw[:, :, 1]
    wsumT = np.concatenate([s_int.T, s12.T, s01.T], axis=1).astype(f32)  # [128,192]

    mlw3 = np.asarray(inputs["manip_lin_w"], f32).reshape(64, 128, 256)
    mlwS = np.concatenate([mlw3[:, 1:127].sum(1), mlw3[:, 0], mlw3[:, 127]],
                          axis=1).astype(f32)                            # [64,768]

    common = {
        "cwE0": cwE0, "cwE1": cwE1, "cwF0": cwF0, "cwF1": cwF1,
        "elw3": np.ascontiguousarray(elw.reshape(256, 128 * 128)).astype(bf16),
        "flw3": np.ascontiguousarray(flw.reshape(256, 128 * 128)).astype(bf16),
        "mlwS": np.ascontiguousarray(mlwS),
        "wsumT": np.ascontiguousarray(wsumT),
        "mcb": np.ascontiguousarray(inputs["manip_conv_b"], f32),
        "elbe": elbe,
        "flbe": flbe,
        "mlb": np.ascontiguousarray(inputs["manip_lin_b"], f32),
        "f2w": np.ascontiguousarray(inputs["friend_lin2_w"], f32),
        "f2b": np.ascontiguousarray(inputs["friend_lin2_b"], f32),
    }
    x = np.asarray(inputs["x"], np.int64)
    idxrow = (V * x[:, 0::2] + x[:, 1::2]).astype(bf16)   # [B, 128], ints < 196
    return [dict(common,
                 idxrowE=np.ascontiguousarray(
                     idxrow[c * BC:(c + 1) * BC].reshape(1, BC * H)))
            for c in range(NCORES)]


def kernel(**inputs):
    nc = _get_nc()
    in_maps = prep_inputs(inputs)
    res = run_bass_kernel_spmd(nc, in_maps, core_ids=list(range(NCORES)))
    return np.concatenate([r["out"] for r in res.results], axis=0)


# revision 25
# speedup vs baseline: 6.3545x; 1.0510x over previous
"""Trainium2 Bass kernel for nn_Network_67388036874689.

Data-parallel over batch: B=256 sharded as 32 samples on each of 8 cores;
all parameters replicated (host-precomposed).

Structure exploited (validated against the reference on host):
  - fog_of_war's greedy scan returns arange(B) -> the permutation is identity.
  - conv2d(3x3, pad=1) on [C, H, 1] spatial input only sees kernel column 1
    -> 1D conv over H with 3 taps.
  - Embedding (V=14) + pair-maxpool + conv compose into per-tap tables
    CW[kh] = pairmax_table @ conv_w[:, :, kh].T  (196 x 256), host-built.
    Device conv = one-hot(pair idx) matmuls against CW with +-1 shifts.
  - Conv bias folds into the following linear's bias (host).
  - The manipulator conv input is constant over h -> the 8192x256 manip
    linear collapses to 3 reduced 64x256 matrices (host-summed over h).

Precision: tables/linears in bf16 (host sim: 2/65536 token flips,
rel err ~1e-4); manipulator path f32/f32r; all psum accumulation f32.
"""

import numpy as np
import ml_dtypes
from contextlib import ExitStack

import concourse.bass as bass
import concourse.bacc as bacc
import concourse.mybir as mybir
import concourse.tile as tile
from concourse.masks import make_identity
from concourse.bass_utils import run_bass_kernel_spmd

F32 = mybir.dt.float32
F32R = mybir.dt.float32r
BF16 = mybir.dt.bfloat16
I32 = mybir.dt.int32
AF = mybir.ActivationFunctionType
ALU = mybir.AluOpType
AX = mybir.AxisListType

NCORES = 8
B = 256
BC = B // NCORES        # 32 samples per core
L = 256
V = 14
EMB = 512
H = L // 2              # 128 pooled positions
NPAIR = V * V           # 196
P0 = 112                # pair-table partition split: 112 + 84
P1 = NPAIR - P0
SW = H + 2              # 130: per-sample padded width in the one-hot tiles
OHW = BC * SW           # 4160
DEBUG_TAPS = False


def _dram_inputs(nc):
    t = {}

    def inp(name, shape, dt):
        t[name] = nc.dram_tensor(name, list(shape), dt, kind="ExternalInput").ap()

    inp("idxrowE", (1, BC * H), BF16)   # host: 14*x[:, 0::2] + x[:, 1::2], flat
    inp("cwE0", (P0, 768), BF16)     # enemy CW tables, col = kh*256 + o
    inp("cwE1", (P1, 768), BF16)
    inp("cwF0", (P0, 768), BF16)
    inp("cwF1", (P1, 768), BF16)
    inp("elw3", (256, 128 * 128), BF16)   # [o, (h, j)]
    inp("flw3", (256, 128 * 128), BF16)
    inp("mlwS", (64, 768), F32R)     # col = v*256 + j, v in (int, h0, hL)
    inp("wsumT", (128, 192), F32R)   # col = v*64 + o
    inp("mcb", (64,), F32)
    inp("elbe", (128,), F32)         # enemy lin bias + folded conv bias
    inp("flbe", (128,), F32)
    inp("mlb", (256,), F32)
    inp("f2w", (128, 14), F32)
    inp("f2b", (14,), F32)
    t["out"] = nc.dram_tensor("out", [BC, 14], F32, kind="ExternalOutput").ap()
    return t


def _tap(nc, io, name, ap):
    if not DEBUG_TAPS:
        return
    t = nc.dram_tensor("tap_" + name, list(ap.shape), ap.dtype,
                       kind="ExternalOutput").ap()
    io["tap_" + name] = t
    nc.gpsimd.dma_start(t, ap)


def build_kernel(nc, tc, ctx):
    io = _dram_inputs(nc)
    consts = ctx.enter_context(tc.tile_pool(name="consts", bufs=1))
    work = ctx.enter_context(tc.tile_pool(name="work", bufs=1))
    wpool = ctx.enter_context(tc.tile_pool(name="wstream", bufs=7))
    ohpool = ctx.enter_context(tc.tile_pool(name="ohpool", bufs=1))
    ppp = ctx.enter_context(tc.tile_pool(name="ppp", bufs=2, space="PSUM"))
    pconv = ctx.enter_context(tc.tile_pool(name="pconv", bufs=4, space="PSUM"))
    plin = ctx.enter_context(tc.tile_pool(name="plin", bufs=1, space="PSUM"))
    psm = ctx.enter_context(tc.tile_pool(name="psm", bufs=1, space="PSUM"))

    def ctile(shape, dt, tag):
        return consts.tile(shape, dt, tag=tag, name=tag)

    def wtile(shape, dt, tag):
        return work.tile(shape, dt, tag=tag, name=tag)

    # ---------------- constants & small weights ----------------
    identF = ctile([128, 128], F32, "identF")
    make_identity(nc, identF)
    iota_i = ctile([128, 1], I32, "iota_i")
    nc.gpsimd.iota(iota_i[:, :], pattern=[[0, 1]], base=0, channel_multiplier=1)
    iota_col = ctile([128, 1], F32, "iota_col")
    nc.vector.tensor_copy(iota_col[:, :], iota_i[:, :])
    ones_row = ctile([1, 128], BF16, "ones_row")
    nc.vector.memset(ones_row[:, :], 1.0)

    def bias_col(dram_vec, n, tag):
        col = ctile([n, 1], F32, tag)
        nc.gpsimd.dma_start(col[:, :], dram_vec)
        return col

    def bias_bcast(dram_vec, rows, width, tag):
        out = ctile([rows, width], F32, tag)
        nc.gpsimd.dma_start(out[:, :], dram_vec[None, :].partition_broadcast(rows))
        return out

    # enemy pair-index row first on the sync HWDGE ring (host-precomputed):
    # the whole front of the kernel needs it
    idxrowE = wtile([1, BC * H], BF16, "idxrowE")
    nc.sync.dma_start(idxrowE[:, :], io["idxrowE"])

    elbeB = bias_bcast(io["elbe"], BC, 128, "elbeB")
    flbeB = bias_bcast(io["flbe"], BC, 128, "flbeB")
    mlbB = bias_bcast(io["mlb"], BC, 256, "mlbB")
    f2bB = bias_bcast(io["f2b"], BC, 14, "f2bB")
    mcb_col = bias_col(io["mcb"], 64, "mcb")

    def load(name, shape, dt):
        t = ctile(shape, dt, name)
        nc.sync.dma_start(t[:, :], io[name])
        return t

    # All HBM loads go on the single sync HWDGE ring in exact consumption
    # order: ring FIFO means the small early loads fully drain before the
    # big weight streams start. (Splitting across rings lets the SDMA
    # engines' packet-granular round-robin starve the small-packet queue:
    # 1.5 KB vs 16 KB packets -> the conv tables took 13+ us to land.)
    # elw3/flw3: [o(256), (h,j)]; piece = [o-half(128), 64 h x 128 j] = 2 MB
    def stream_weights(dram):
        pieces = []
        for half in range(2):
            for hb in range(2):
                p = wpool.tile([128, 64 * 128], BF16, tag="wp", name="wp")
                nc.sync.dma_start(
                    p[:, :], dram[half * 128:(half + 1) * 128,
                                  hb * 8192:(hb + 1) * 8192])
                pieces.append(p)
        return pieces

    cwE0 = load("cwE0", [P0, 768], BF16)
    cwE1 = load("cwE1", [P1, 768], BF16)
    elwP = stream_weights(io["elw3"])
    wsumT = load("wsumT", [128, 192], F32R)
    mlwS = load("mlwS", [64, 768], F32R)
    cwF0 = load("cwF0", [P0, 768], BF16)
    cwF1 = load("cwF1", [P1, 768], BF16)
    flwP = stream_weights(io["flw3"])
    w2sb = load("f2w", [128, 14], F32)

    # ---------------- stage helpers ----------------
    def build_oh(idxrow, tag):
        """One-hot over the pair-idx row [1, 4096] (col s*128+h), padded
        layout: col s*130 + 1 + h holds [idx[s,h] == t]; cols s*130 and
        s*130+129 are zero (conv boundary)."""
        oh0 = ohpool.tile([P0, OHW], BF16, tag="oh0", name=f"oh0{tag}")
        oh1 = ohpool.tile([P1, OHW], BF16, tag="oh1", name=f"oh1{tag}")
        nc.vector.memset(oh0[:, 0:OHW:SW], 0.0)
        nc.vector.memset(oh0[:, SW - 1:OHW:SW], 0.0)
        nc.gpsimd.memset(oh1[:, 0:OHW:SW], 0.0)
        nc.gpsimd.memset(oh1[:, SW - 1:OHW:SW], 0.0)
        for blk in range(8):
            pp = ppp.tile([P0, 512], F32, tag="pp", name="pp")
            nc.tensor.matmul(pp[:, :], ones_row[:, 0:P0],
                             idxrow[:, blk * 512:(blk + 1) * 512],
                             start=True, stop=True)
            src = pp[:, :].rearrange("p (s w) -> p s w", w=128)
            dst0 = oh0[:, blk * 4 * SW:(blk + 1) * 4 * SW] \
                .rearrange("p (s w) -> p s w", w=SW)[:, :, 1:129]
            nc.vector.tensor_scalar(dst0, src, iota_col[0:P0, :], None,
                                    ALU.is_equal)
            dst1 = oh1[:, blk * 4 * SW:(blk + 1) * 4 * SW] \
                .rearrange("p (s w) -> p s w", w=SW)[:, :, 1:129]
            nc.vector.tensor_scalar(dst1, src[0:P1], float(P0),
                                    iota_col[0:P1, :], ALU.subtract,
                                    ALU.is_equal)
        return oh0, oh1

    def conv_apply(oh0, oh1, cw0, cw1, tag):
        """y[o, (s,h)] = sum_kh CW_kh[idx[h+kh-1], o]; acts as 2 halves
        [128 o', 32*128 (s,h)] bf16."""
        acts = [wtile([128, BC * H], BF16, f"acts{tag}{oc}") for oc in range(2)]
        for oc in range(2):
            for blk in range(8):
                cp = pconv.tile([128, 512], F32, tag="cp", name="cp")
                n = 0
                for cw, oh, npart in ((cw0, oh0, P0), (cw1, oh1, P1)):
                    for kh in range(3):
                        lhsT = cw[:, kh * 256 + oc * 128:
                                  kh * 256 + (oc + 1) * 128]
                        rhs = oh[:, blk * 4 * SW:(blk + 1) * 4 * SW] \
                            .rearrange("p (s w) -> p s w", w=SW)[:, :, kh:kh + 128]
                        nc.tensor.matmul(cp[:, :], lhsT, rhs,
                                         start=(n == 0), stop=(n == 5))
                        n += 1
                dst = acts[oc][:, blk * 512:(blk + 1) * 512]
                if blk % 2 == 0:
                    nc.scalar.activation(dst, cp[:, :], AF.Copy)
                else:
                    nc.vector.tensor_copy(dst, cp[:, :])
        return acts

    def big_linear(acts, pieces, tag):
        """lp[s, j] = sum_{o,h} acts[o][:, s*128+h] * W[(o,h), j]"""
        lp = plin.tile([BC, 128], F32, tag="lp", name=f"lp{tag}")
        for half in range(2):
            for h in range(128):
                piece = pieces[half * 2 + h // 64]
                lhsT = acts[half][:, h:h + (BC - 1) * 128 + 1:128]
                rhs = piece[:, (h % 64) * 128:(h % 64 + 1) * 128]
                nc.tensor.matmul(lp[:, :], lhsT, rhs,
                                 start=(half == 0 and h == 0),
                                 stop=(half == 1 and h == 127))
        return lp

    # ---------------- enemy branch ----------------
    ohE0, ohE1 = build_oh(idxrowE, "E")
    actsE = conv_apply(ohE0, ohE1, cwE0, cwE1, "E")
    _tap(nc, io, "actsE0", actsE[0][:, :])
    lpE = big_linear(actsE, elwP, "E")

    logitsE = wtile([BC, 128], F32, "logitsE")
    nc.vector.tensor_tensor(logitsE[:, :], lpE[:, :], elbeB[:, :], ALU.add)
    _tap(nc, io, "logitsE", logitsE[:, :])
    nmxE = wtile([BC, 1], F32, "nmxE")
    nc.vector.reduce_max(nmxE[:, :], logitsE[:, :], AX.X, negate=True)
    ExE = wtile([BC, 128], F32, "ExE")
    nc.scalar.activation(ExE[:, :], logitsE[:, :], AF.Exp, bias=nmxE[:, :])
    smE = wtile([BC, 1], F32, "smE")
    nc.vector.reduce_sum(smE[:, :], ExE[:, :], AX.X)
    rsE = wtile([BC, 1], F32, "rsE")
    nc.vector.reciprocal(rsE[:, :], smE[:, :])
    eout = wtile([BC, 128], F32, "eout")
    nc.vector.tensor_scalar(eout[:, :], ExE[:, :], rsE[:, :], None, ALU.mult)

    tpv = psm.tile([128, BC], F32, tag="sm", name="tpv")
    nc.tensor.transpose(tpv[:, :], eout[:, :], identF[0:BC, 0:BC])
    vT = wtile([128, BC], F32R, "vT")
    nc.vector.tensor_copy(vT[:, :], tpv[:, :])
    _tap(nc, io, "vT", vT[:, :])

    # ---------------- manipulator ----------------
    cxs = {}
    for i, v in enumerate(("int", "h0", "hL")):
        cx = psm.tile([64, BC], F32, tag="sm", name=f"cx{v}")
        nc.tensor.matmul(cx[:, :], wsumT[:, i * 64:(i + 1) * 64], vT[:, :],
                         start=True, stop=True)
        cxs[v] = wtile([64, BC], F32R, f"cxs_{v}")
        nc.scalar.activation(cxs[v][:, :], cx[:, :], AF.Relu, bias=mcb_col[:, :])
    mp = plin.tile([BC, 256], F32, tag="lp", name="mp")
    for i, v in enumerate(("int", "h0", "hL")):
        nc.tensor.matmul(mp[:, :], cxs[v][:, :], mlwS[:, i * 256:(i + 1) * 256],
                         start=(i == 0), stop=(i == 2))
    m_sb = wtile([BC, 256], F32, "m_sb")
    nc.vector.tensor_tensor(m_sb[:, :], mp[:, :], mlbB[:, :], ALU.add)
    _tap(nc, io, "m", m_sb[:, :])

    # tokens = floor(|m|*100) mod 14; pair idx = 14*even + odd
    # floor via the 2^23 magic-number trick; mod 14 via 2 conditional subtracts
    tt = wtile([BC, 256], F32, "tt")
    nc.scalar.activation(tt[:, :], m_sb[:, :], AF.Abs, scale=100.0)
    fr = wtile([BC, 256], F32, "fr")
    nc.vector.tensor_scalar(fr[:, :], tt[:, :], 8388607.5, 8388608.0,
                            ALU.add, ALU.subtract)
    ti = wtile([BC, 256], F32, "ti")
    nc.vector.tensor_scalar(ti[:, :], fr[:, :], float(V), None, ALU.is_ge)
    t1 = wtile([BC, 256], F32, "t1")
    nc.vector.scalar_tensor_tensor(t1[:, :], ti[:, :], -float(V), fr[:, :],
                                   ALU.mult, ALU.add)
    t2 = wtile([BC, 256], F32, "t2")
    nc.vector.tensor_scalar(t2[:, :], t1[:, :], float(V), None, ALU.is_ge)
    tok = wtile([BC, 256], F32, "tok")
    nc.vector.scalar_tensor_tensor(tok[:, :], t2[:, :], -float(V), t1[:, :],
                                   ALU.mult, ALU.add)
    _tap(nc, io, "tok", tok[:, :])
    idxF = wtile([BC, H], BF16, "idxF")
    nc.vector.scalar_tensor_tensor(idxF[:, :], tok[:, 0:256:2], float(V),
                                   tok[:, 1:256:2], ALU.mult, ALU.add)
    idxrowF = wtile([1, BC * H], BF16, "idxrowF")
    nc.gpsimd.dma_start(idxrowF[:, :], idxF[:, :])

    # ---------------- friend branch ----------------
    ohF0, ohF1 = build_oh(idxrowF, "F")
    actsF = conv_apply(ohF0, ohF1, cwF0, cwF1, "F")
    lpF = big_linear(actsF, flwP, "F")
    fsb = wtile([BC, 128], F32, "fsb")
    nc.vector.tensor_tensor(fsb[:, :], lpF[:, :], flbeB[:, :], ALU.add)

    tpf = psm.tile([128, BC], F32, tag="sm", name="tpf")
    nc.tensor.transpose(tpf[:, :], fsb[:, :], identF[0:BC, 0:BC])
    fT = wtile([128, BC], F32, "fT")
    nc.vector.tensor_copy(fT[:, :], tpf[:, :])
    f2 = psm.tile([BC, 14], F32, tag="sm", name="f2")
    nc.tensor.matmul(f2[:, :], fT[:, :], w2sb[:, :], start=True, stop=True)
    logits = wtile([BC, 14], F32, "logits")
    nc.vector.tensor_tensor(logits[:, :], f2[:, :], f2bB[:, :], ALU.add)
    nmx = wtile([BC, 1], F32, "nmx")
    nc.vector.reduce_max(nmx[:, :], logits[:, :], AX.X, negate=True)
    ex = wtile([BC, 14], F32, "ex")
    nc.scalar.activation(ex[:, :], logits[:, :], AF.Exp, bias=nmx[:, :])
    sm = wtile([BC, 1], F32, "sm")
    nc.vector.reduce_sum(sm[:, :], ex[:, :], AX.X)
    rs = wtile([BC, 1], F32, "rs")
    nc.vector.reciprocal(rs[:, :], sm[:, :])
    outt = wtile([BC, 14], F32, "outt")
    nc.vector.tensor_scalar(outt[:, :], ex[:, :], rs[:, :], None, ALU.mult)
    nc.sync.dma_start(io["out"], outt[:, :])


_CACHE = {}


def _get_nc():
    if "nc" not in _CACHE:
        nc = bacc.Bacc("TRN2", target_bir_lowering=False, debug=False,
                       num_devices=NCORES)
        with tile.TileContext(nc) as tc:
            with ExitStack() as ctx:
                build_kernel(nc, tc, ctx)
        nc.compile()
        _CACHE["nc"] = nc
    return _CACHE["nc"]


def prep_inputs(inputs):
    """Host-side composition + shard. Returns list of 8 in_maps."""
    f32 = np.float32
    bf16 = ml_dtypes.bfloat16

    def cw_tables(emb, cw_full):
        emb = np.asarray(emb, f32)
        cw = np.ascontiguousarray(np.asarray(cw_full, f32)[:, :, :, 1])  # [O,I,3]
        t0, t1 = np.meshgrid(np.arange(V), np.arange(V), indexing="ij")
        table = np.maximum(emb[t0.ravel()], emb[t1.ravel()])            # [196,512]
        cwc = np.concatenate([table @ cw[:, :, kh].T for kh in range(3)],
                             axis=1).astype(bf16)                        # [196,768]
        return np.ascontiguousarray(cwc[:P0]), np.ascontiguousarray(cwc[P0:])

    cwE0, cwE1 = cw_tables(inputs["enemy_emb"], inputs["enemy_conv_w"])
    cwF0, cwF1 = cw_tables(inputs["friend_emb"], inputs["friend_conv_w"])

    elw = np.asarray(inputs["enemy_lin_w"], f32)
    flw = np.asarray(inputs["friend_lin1_w"], f32)
    elbe = (np.asarray(inputs["enemy_lin_b"], f32)
            + np.einsum("o,ohj->j", np.asarray(inputs["enemy_conv_b"], f32),
                        elw.reshape(256, 128, 128), optimize=True)).astype(f32)
    flbe = (np.asarray(inputs["friend_lin1_b"], f32)
            + np.einsum("o,ohj->j", np.asarray(inputs["friend_conv_b"], f32),
                        flw.reshape(256, 128, 128), optimize=True)).astype(f32)

    mcw = np.asarray(inputs["manip_conv_w"], f32)[:, :, :, 1]  # [64,128,3]
    s_int = mcw.sum(2)
    s12 = mcw[:, :, 1] + mcw[:, :, 2]
    s01 = mcw[:, :, 0] + mcw[:, :, 1]
    wsumT = np.concatenate([s_int.T, s12.T, s01.T], axis=1).astype(f32)  # [128,192]

    mlw3 = np.asarray(inputs["manip_lin_w"], f32).reshape(64, 128, 256)
    mlwS = np.concatenate([mlw3[:, 1:127].sum(1), mlw3[:, 0], mlw3[:, 127]],
                          axis=1).astype(f32)                            # [64,768]

    common = {
        "cwE0": cwE0, "cwE1": cwE1, "cwF0": cwF0, "cwF1": cwF1,
        "elw3": np.ascontiguousarray(elw.reshape(256, 128 * 128)).astype(bf16),
        "flw3": np.ascontiguousarray(flw.reshape(256, 128 * 128)).astype(bf16),
        "mlwS": np.ascontiguousarray(mlwS),
        "wsumT": np.ascontiguousarray(wsumT),
        "mcb": np.ascontiguousarray(inputs["manip_conv_b"], f32),
        "elbe": elbe,
        "flbe": flbe,
        "mlb": np.ascontiguousarray(inputs["manip_lin_b"], f32),
        "f2w": np.ascontiguousarray(inputs["friend_lin2_w"], f32),
        "f2b": np.ascontiguousarray(inputs["friend_lin2_b"], f32),
    }
    x = np.asarray(inputs["x"], np.int64)
    idxrow = (V * x[:, 0::2] + x[:, 1::2]).astype(bf16)   # [B, 128], ints < 196
    return [dict(common,
                 idxrowE=np.ascontiguousarray(
                     idxrow[c * BC:(c + 1) * BC].reshape(1, BC * H)))
            for c in range(NCORES)]


def kernel(**inputs):
    nc = _get_nc()
    in_maps = prep_inputs(inputs)
    res = run_bass_kernel_spmd(nc, in_maps, core_ids=list(range(NCORES)))
    return np.concatenate([r["out"] for r in res.results], axis=0)
